# revision 1
# baseline (speedup 1.0000x reference)
"""Trainium2 Bass kernel for causal multi-head attention (B=2, L=2048, D=2048,
H=16 heads, DH=128), sharded over 8 NeuronCores.

Sharding: core c handles batch b=c//4 and head-group g=c%4 (4 heads = 512
features). The only cross-core communication is a per-head-chunk fp16
AllGather of attention outputs within each 4-core batch group.

Precision scheme (fp16 datapath, fp32 PSUM accumulation):
- Softmax temperature is sqrt(128) (reference multiplies scores by
  sqrt(d_head)), so score errors are amplified ~11x before exp.
- Q/K projections run compensated (~22 effective bits): fp16 hi*hi main term
  + the two cross terms (lo*hi, hi*lo) in fp8e4m3 DoubleRow, accumulated in a
  second PSUM at scale 2^17 and folded in during evacuation (ACT).
- qh/kh are re-split on device into fp16 hi + e5m2 (hi,lo) planes at natural
  scale; scores S = hi*hi fp16 matmul + ONE fp8e5m2 DoubleRow matmul
  computing both cross terms (qhi*klo + qlo*khi), accumulating directly into
  the same PSUM (no scale fold needed since e5m2 holds natural scale).
- V path, P = softmax(S), attention output, and Wo run in plain fp16.

Layout/overlap notes:
- All x/w DRAM operands are staged host-side in exactly the SBUF tile order
  (partition-major), so every load is 128 contiguous descriptors instead of
  2048 — the DMA issue queue (SP.SEQ) was the v2 bottleneck.
- P^T is produced by DMA-engine transposes (SBUF->SBUF, per block in
  halves, SP queue only), replacing PE transposes + PSUM evacuation
  copies. Gather traffic rides the ACT queue so it round-robins fairly
  against the transposes at the shared HWDGE.
- Attention runs a 3-stage software pipeline (S matmuls of block n, row
  max + exp of block n-1, stats/scale/transpose/AV of block n-2) with
  depth adapted to the 6-bank score-PSUM budget. The two O^T accumulators
  must each own a full PSUM bank: two accumulation groups sharing a bank
  corrupt each other via bank-granular has_written clears (HW-verified).
- Gather-emulation copies are queued and drained one per even block of
  the following head; Wo weights prefetch during attention and the
  gathered activations stream in 16 one-block slices through a 4-deep
  ring, so the output projection starts after 1 MB instead of 4 MB.
- Known dead ends (all measured slower): Pool-engine pscales or per-chunk
  transposes (chain latency), head interleaving / deeper pipelines (PE
  FIFO head-of-line inversions), mid-attention gather injection (DMA
  engine is effectively serial; displaces transposes), front-loading the
  stats chain (delays next block's maxes -> PSUM recycling).
  tensor_tensor_reduce accum_out has broken dep tracking (races).
"""
import sys

sys.path.insert(0, "/opt/trn_rl_repo")

import numpy as np
import ml_dtypes

B, L, D, H = 2, 2048, 2048, 16
DH = D // H          # 128
G = 4                # head-groups (tensor-parallel degree per batch)
HPG = H // G         # heads per group = 4
FPG = HPG * DH       # features per group = 512
P = 128
SCALE = float(DH) ** 0.5
KC = D // P          # 16 contraction chunks
IB = L // P          # 16 seq blocks of 128
IPANEL = 512         # projection moving-dim panel
NPAN = L // IPANEL   # 4
KP = D // 256        # 8 DoubleRow contraction chunks

_COMPILED = None


def _build(variant="main"):
    import concourse.bacc as bacc
    import concourse.tile as tile
    from concourse import mybir
    from contextlib import ExitStack

    F32 = mybir.dt.float32
    F16 = mybir.dt.float16
    F8 = mybir.dt.float8e4
    F8E5 = mybir.dt.float8e5
    AX = mybir.AxisListType
    OP = mybir.AluOpType
    ACTF = mybir.ActivationFunctionType
    DRM = mybir.MatmulPerfMode.DoubleRow

    nc = bacc.Bacc("TRN2", target_bir_lowering=False, debug=False, num_devices=8)

    # ---- DRAM I/O (all x/w pre-arranged host-side in tile order) ----
    xqh = nc.dram_tensor("xqh", [P, NPAN, KC, IPANEL], F16, kind="ExternalInput")
    xkh = nc.dram_tensor("xkh", [P, NPAN, KC, IPANEL], F16, kind="ExternalInput")
    xvh = nc.dram_tensor("xvh", [P, NPAN, KC, IPANEL], F16, kind="ExternalInput")
    wqh = nc.dram_tensor("wqh", [P, KC, FPG], F16, kind="ExternalInput")
    wkh = nc.dram_tensor("wkh", [P, KC, FPG], F16, kind="ExternalInput")
    wvh = nc.dram_tensor("wvh", [P, KC, FPG], F16, kind="ExternalInput")
    xq8h = nc.dram_tensor("xq8h", [P, NPAN, KP, 2, IPANEL], F8, kind="ExternalInput")
    xq8l = nc.dram_tensor("xq8l", [P, NPAN, KP, 2, IPANEL], F8, kind="ExternalInput")
    xk8h = nc.dram_tensor("xk8h", [P, NPAN, KP, 2, IPANEL], F8, kind="ExternalInput")
    xk8l = nc.dram_tensor("xk8l", [P, NPAN, KP, 2, IPANEL], F8, kind="ExternalInput")
    wq8h = nc.dram_tensor("wq8h", [P, KP, 2, FPG], F8, kind="ExternalInput")
    wq8l = nc.dram_tensor("wq8l", [P, KP, 2, FPG], F8, kind="ExternalInput")
    wk8h = nc.dram_tensor("wk8h", [P, KP, 2, FPG], F8, kind="ExternalInput")
    wk8l = nc.dram_tensor("wk8l", [P, KP, 2, FPG], F8, kind="ExternalInput")
    woT = nc.dram_tensor("woT", [P, KC, FPG], F16, kind="ExternalInput")
    bq = nc.dram_tensor("bq", [P, HPG], F32, kind="ExternalInput")
    bk = nc.dram_tensor("bk", [P, HPG], F32, kind="ExternalInput")
    bvb = nc.dram_tensor("bvb", [P, FPG], F32, kind="ExternalInput")
    bob = nc.dram_tensor("bob", [P, FPG], F32, kind="ExternalInput")
    maskh = nc.dram_tensor("maskh", [P, P], F16, kind="ExternalInput")
    identd = nc.dram_tensor("identd", [P, P], F16, kind="ExternalInput")
    out = nc.dram_tensor("out", [L, FPG], F32, kind="ExternalOutput")
    if variant == "timing":
        chain = nc.dram_tensor("chain", [1, 8], F32, kind="ExternalInput")
        dummy = nc.dram_tensor("chaino", [1, 8], F32, kind="ExternalOutput")

    with tile.TileContext(nc) as tc:
        with ExitStack() as ctx:
            consts = ctx.enter_context(tc.tile_pool(name="consts", bufs=1))

            maskh_t = consts.tile([P, P], F16)
            id_t = consts.tile([P, P], F16)
            bq_t = consts.tile([P, HPG], F32)
            nc.sync.dma_start(bq_t[:], bq[:])
            bk_t = consts.tile([P, HPG], F32)
            nc.sync.dma_start(bk_t[:], bk[:])
            bvb_t = consts.tile([P, FPG], F32)
            nc.scalar.dma_start(bvb_t[:], bvb[:])
            bob_t = consts.tile([P, FPG], F32)
            if variant == "timing":
                ch_t = consts.tile([1, 8], F32)
                nc.sync.dma_start(ch_t[:], chain[:])
                nc.sync.dma_start(dummy[:], ch_t[:])

            NREP = {"x4": 4, "x2": 2, "x2nc": 2}.get(variant, 1)
            for _rep in range(NREP):
                ag_outs = []
                with tc.tile_pool(name="qkv", bufs=1) as qkv:
                    qth = qkv.tile([P, HPG, L], F16)       # (d, head, seq) hi
                    kth = qkv.tile([P, HPG, L], F16)
                    q8t = qkv.tile([P, HPG, 2, L], F8E5)   # planes: 0=hi 1=lo
                    k8t = qkv.tile([P, HPG, 2, L], F8E5)   # planes: 0=lo 1=hi
                    vt = qkv.tile([P, IB, FPG], F16)       # (seq%128, blk, feat)

                    # ---- phase 1: projections ----
                    with tc.tile_pool(name="wpool", bufs=2) as wpool, \
                         tc.tile_pool(name="xpool", bufs=2) as xpool, \
                         tc.tile_pool(name="tpool", bufs=2) as tpool, \
                         tc.tile_pool(name="ppsum", bufs=3, space="PSUM") as ppsum, \
                         tc.tile_pool(name="vpsum", bufs=2, space="PSUM") as vpsum:

                        # Q and K projections -> (feature, seq) fp16 hi +
                        # e5m2 (hi, lo) planes. Main term fp16; corrections
                        # (lo*hi + hi*lo) in fp8e4m3 DoubleRow at scale 2^17.
                        def proj_qk(xh_d, x8h_d, x8l_d, wh_d, w8h_d, w8l_d,
                                    bias_t, dh, d8, lo_first, first=False):
                            wh_t = wpool.tile([P, KC, FPG], F16, tag="w")
                            nc.sync.dma_start(wh_t[:, :KC // 2], wh_d[:, :KC // 2])
                            w8h_t = wpool.tile([P, KP, 2, FPG], F8, tag="w8")
                            w8l_t = wpool.tile([P, KP, 2, FPG], F8, tag="w8")
                            for ip in range(NPAN):
                                isl = slice(ip * IPANEL, (ip + 1) * IPANEL)
                                xh_t = xpool.tile([P, KC, IPANEL], F16, tag="x")
                                nc.sync.dma_start(xh_t[:], xh_d[:, ip])
                                if ip == 0:
                                    nc.sync.dma_start(wh_t[:, KC // 2:],
                                                      wh_d[:, KC // 2:])
                                    nc.sync.dma_start(w8h_t[:], w8h_d[:])
                                    nc.sync.dma_start(w8l_t[:], w8l_d[:])
                                x8h_t = xpool.tile([P, KP, 2, IPANEL], F8, tag="x8h")
                                nc.sync.dma_start(x8h_t[:], x8h_d[:, ip])
                                x8l_t = xpool.tile([P, KP, 2, IPANEL], F8, tag="x8l")
                                nc.sync.dma_start(x8l_t[:], x8l_d[:, ip])
                                for fc in range(HPG):
                                    fsl = slice(fc * P, (fc + 1) * P)
                                    ps = ppsum.tile([P, IPANEL], F32, tag="pp")
                                    for kc in range(KC):
                                        nc.tensor.matmul(
                                            ps[:], wh_t[:, kc, fsl], xh_t[:, kc, :],
                                            start=(kc == 0), stop=(kc == KC - 1))
                                    psb = ppsum.tile([P, IPANEL], F32, tag="pb")
                                    for kp in range(KP):
                                        nc.tensor.matmul(
                                            psb[:], w8h_t[:, kp, :, fsl],
                                            x8l_t[:, kp, :, :],
                                            start=(kp == 0), stop=False,
                                            perf_mode=DRM)
                                        nc.tensor.matmul(
                                            psb[:], w8l_t[:, kp, :, fsl],
                                            x8h_t[:, kp, :, :],
                                            start=False, stop=(kp == KP - 1),
                                            perf_mode=DRM)
                                    # evacuation: ACT folds corr, Pool adds
                                    # main + writes fp16 hi, DVE writes the
                                    # e5m2 planes (hi plane index depends on
                                    # DR pairing: q planes (hi,lo), k (lo,hi))
                                    tmp = tpool.tile([P, IPANEL], F32, tag="t")
                                    nc.scalar.activation(
                                        tmp[:], psb[:], ACTF.Identity,
                                        bias=bias_t[:, fc:fc + 1],
                                        scale=2.0 ** -17)
                                    tmp2 = tpool.tile([P, IPANEL], F32, tag="t2")
                                    nc.vector.tensor_tensor(
                                        tmp2[:], tmp[:], ps[:], op=OP.add)
                                    nc.gpsimd.tensor_copy(dh[:, fc, isl], tmp2[:])
                                    hi_pl, lo_pl = (1, 0) if lo_first else (0, 1)
                                    nc.vector.tensor_copy(
                                        d8[:, fc, hi_pl, isl], tmp2[:])
                                    nc.vector.tensor_tensor(
                                        d8[:, fc, lo_pl, isl], tmp2[:],
                                        dh[:, fc, isl], op=OP.subtract)

                        # Order V, Q, K: V needs the least data (fastest
                        # start), K's outputs are the freshest when attention
                        # starts, and V's PSUM banks (reused by the attention
                        # score pool) are long released by then.
                        wv_t = wpool.tile([P, KC, FPG], F16, tag="w")
                        for ip in range(NPAN):
                            xv_t = xpool.tile([P, KC, IPANEL], F16, tag="x")
                            if ip == 0:
                                # interleave weight/x eighths so the first
                                # matmul's operands land first
                                for xc in range(8):
                                    xsl = slice(xc * KC // 8,
                                                (xc + 1) * KC // 8)
                                    nc.sync.dma_start(xv_t[:, xsl],
                                                      xvh[:, ip, xsl])
                                    nc.sync.dma_start(wv_t[:, xsl],
                                                      wvh[:, xsl])
                            else:
                                nc.sync.dma_start(xv_t[:], xvh[:, ip])
                            for sub in range(IPANEL // P):
                                ib = ip * (IPANEL // P) + sub
                                ps = vpsum.tile([P, FPG], F32, tag="pv")
                                for kc in range(KC):
                                    nc.tensor.matmul(
                                        ps[:],
                                        xv_t[:, kc, sub * P:(sub + 1) * P],
                                        wv_t[:, kc, :],
                                        start=(kc == 0), stop=(kc == KC - 1))
                                nc.vector.tensor_tensor(
                                    vt[:, ib, :], ps[:], bvb_t[:], op=OP.add)

                        if _rep == 0:
                            nc.scalar.dma_start(maskh_t[:], maskh[:])
                            nc.scalar.dma_start(id_t[:], identd[:])
                            nc.scalar.dma_start(bob_t[:], bob[:])
                        proj_qk(xqh, xq8h, xq8l, wqh, wq8h, wq8l, bq_t,
                                qth, q8t, False)
                        proj_qk(xkh, xk8h, xk8l, wkh, wk8h, wk8l, bk_t,
                                kth, k8t, True)

                    # ---- phase 2+3 tiles: Wo weights prefetched during
                    # attention; gathered activations stream in phase 3 ----
                    with tc.tile_pool(name="ph3", bufs=1) as ph3:
                        wo_t = ph3.tile([P, KC, FPG], F16, name=f"wo{_rep}")
                        nc.scalar.dma_start(wo_t[:, :KC // 2], woT[:, :KC // 2])
                        nc.scalar.dma_start(wo_t[:, KC // 2:], woT[:, KC // 2:])

                        with tc.tile_pool(name="otpool", bufs=1) as otpool, \
                             tc.tile_pool(name="spsum", bufs=6, space="PSUM") as spsum, \
                             tc.tile_pool(name="opsum", bufs=2, space="PSUM") as opsum, \
                             tc.tile_pool(name="ppool", bufs=6) as ppool, \
                             tc.tile_pool(name="ptpool", bufs=5) as ptpool, \
                             tc.tile_pool(name="stats", bufs=8) as stats, \
                             tc.tile_pool(name="dramio", bufs=1, space="DRAM") as dramio:

                            ots = [otpool.tile([P, L], F16, name=f"ot{hh}")
                                   for hh in range(HPG)]
                            # pending gathered-activation strip loads,
                            # drained into the next head's block loop
                            at_pending = []

                            def emit_S(h, ib):
                                nj = (ib + 1) * P
                                nch = (nj + 511) // 512
                                isl = slice(ib * P, (ib + 1) * P)
                                mpart = stats.tile([P, 4], F32, tag="mp",
                                                   name=f"mp{h}_{ib}")
                                p_sb = ppool.tile([P, L], F16, tag="p",
                                                  name=f"p{h}_{ib}")
                                lpart = stats.tile([P, 4], F32, tag="lp",
                                                   name=f"lp{h}_{ib}")
                                chunks = []
                                for jc in range(nch):
                                    w = min(512, nj - jc * 512)
                                    jsl = slice(jc * 512, jc * 512 + w)
                                    diag = jc == nch - 1
                                    ps = spsum.tile([P, 512], F32, tag="s",
                                                    name=f"sps{h}_{ib}_{jc}")
                                    nc.tensor.matmul(
                                        ps[:, :w], qth[:, h, isl], kth[:, h, jsl],
                                        start=True, stop=False)
                                    nc.tensor.matmul(
                                        ps[:, :w], q8t[:, h, :, isl],
                                        k8t[:, h, :, jsl],
                                        start=False, stop=not diag,
                                        perf_mode=DRM)
                                    if diag:
                                        # causal mask on the diagonal block,
                                        # accumulated on the PE: += I.T @ mask
                                        nc.tensor.matmul(
                                            ps[:, w - P:w], id_t[:], maskh_t[:],
                                            start=False, stop=True)
                                    chunks.append((ps, w, jsl, jc))
                                return p_sb, mpart, lpart, chunks

                            def emit_S_stats(h, ib, p_sb, mpart, lpart,
                                             chunks):
                                # scores arrive pre-scaled (host folds
                                # sqrt(scale) into Wq/Wk): negated chunk
                                # max IS the exp bias
                                for ps, w, jsl, jc in chunks:
                                    nc.vector.reduce_max(
                                        mpart[:, jc:jc + 1], ps[:, :w],
                                        axis=AX.X, negate=True)
                                    nc.scalar.activation(
                                        p_sb[:, jsl], ps[:, :w],
                                        ACTF.Exp, bias=mpart[:, jc:jc + 1],
                                        scale=1.0,
                                        accum_out=lpart[:, jc:jc + 1])

                            def emit_softmax_av(h, ib, p_sb, mpart, lpart,
                                                chunks):
                                nj = (ib + 1) * P
                                nch = (nj + 511) // 512
                                isl = slice(ib * P, (ib + 1) * P)
                                rmin = stats.tile([P, 1], F32, tag="nm",
                                                  name=f"nm{h}_{ib}")
                                nc.vector.tensor_reduce(
                                    rmin[:], mpart[:, :nch], axis=AX.X, op=OP.min)
                                # per-chunk correction c = exp(m_jc - m)
                                cfac = stats.tile([P, 4], F32, tag="cf",
                                                  name=f"cf{h}_{ib}")
                                nc.scalar.activation(
                                    cfac[:, :nch], mpart[:, :nch],
                                    ACTF.Exp, bias=rmin[:], scale=-1.0)
                                lw = stats.tile([P, 4], F32, tag="lw",
                                                name=f"lw{h}_{ib}")
                                nc.vector.tensor_tensor(
                                    lw[:, :nch], cfac[:, :nch], lpart[:, :nch],
                                    op=OP.mult)
                                lsum = stats.tile([P, 1], F32, tag="ls",
                                                  name=f"ls{h}_{ib}")
                                nc.vector.reduce_sum(lsum[:], lw[:, :nch],
                                                     axis=AX.X)
                                rinv = stats.tile([P, 1], F32, tag="ri",
                                                  name=f"ri{h}_{ib}")
                                nc.vector.reciprocal(rinv[:], lsum[:])
                                # P_jc *= c_jc * rinv (DVE 4x); transposes
                                # in halves so first-half AV matmuls overlap
                                # the second half's transpose latency
                                for jc in range(nch):
                                    w = min(512, nj - jc * 512)
                                    jsl = slice(jc * 512, jc * 512 + w)
                                    nc.vector.tensor_scalar(
                                        p_sb[:, jsl], p_sb[:, jsl],
                                        cfac[:, jc:jc + 1], rinv[:],
                                        op0=OP.mult, op1=OP.mult)
                                pt_sb = ptpool.tile([P, IB, P], F16, tag="ptsb",
                                                    name=f"ptsb{h}_{ib}")
                                o_tile = opsum.tile([P, P], F32, tag="o",
                                                    name=f"o{h}_{ib}")
                                o_ps = o_tile[:]
                                hsplit = [(0, nch)] if nch <= 2 else \
                                    [(0, 2), (2, nch)]
                                for hp, (c0, c1) in enumerate(hsplit):
                                    j0 = c0 * 4
                                    j1 = min(c1 * 4, ib + 1)
                                    nc.sync.dma_start_transpose(
                                        pt_sb[:, j0:j1, :],
                                        p_sb[:, j0 * P:j1 * P])
                                for jb in range(ib + 1):
                                    nc.tensor.matmul(
                                        o_ps, vt[:, jb, h * P:(h + 1) * P],
                                        pt_sb[:, jb, :],
                                        start=(jb == 0), stop=(jb == ib))
                                nc.vector.tensor_copy(ots[h][:, isl], o_ps)

                            def emit_gather(h):
                                ag_in = dramio.tile([P, L], F16, tag=f"agin{h}",
                                                    name=f"agin{h}")
                                nc.scalar.dma_start(ag_in[:], ots[h][:])
                                ag_out = dramio.tile([G, P, L], F16,
                                                     tag=f"agout{h}",
                                                     name=f"agout{h}")
                                if variant in ("nocoll", "x2nc"):
                                    # strip the gather-emulation copies so
                                    # they don't burst ahead of the next
                                    # head's P^T transposes at the DMA engines
                                    for gg in range(G):
                                        def cp(gg=gg, ag_out=ag_out,
                                               ag_in=ag_in):
                                            nc.scalar.dma_start(
                                                ag_out[gg], ag_in[:])
                                        at_pending.append(cp)
                                else:
                                    nc.gpsimd.collective_compute(
                                        "AllGather", OP.bypass,
                                        replica_groups=[[0, 1, 2, 3], [4, 5, 6, 7]],
                                        ins=[ag_in[:].opt()], outs=[ag_out[:].opt()])
                                ag_outs.append(ag_out)

                            def drain_at(n):
                                for _ in range(n):
                                    if not at_pending:
                                        return
                                    at_pending.pop(0)()

                            # 2-block software pipeline (crossing head
                            # boundaries): S of blocks n+1, n+2 are emitted
                            # before softmax/AV of block n so the PE always
                            # has score matmuls queued while the softmax
                            # chain (DVE/ACT/DMA transpose) drains
                            # 3-stage pipeline: S matmuls of block n,
                            # stats (max+exp) of block n-1, softmax/AV of
                            # block n-2 — keeps each softmax's pscales at
                            # the DVE FIFO head (never behind future maxes)
                            pend = []

                            def drain_one():
                                e = pend.pop(0)
                                emit_softmax_av(*e)
                                if e[1] % 2 == 0:
                                    drain_at(1)
                                if e[1] == IB - 1:
                                    emit_gather(e[0])

                            def nch_of(e):
                                return (e[1] * P + P + 511) // 512

                            for h in range(HPG):
                                for ib in range(IB):
                                    pend.append((h, ib, *emit_S(h, ib)))
                                    if len(pend) >= 2:
                                        emit_S_stats(*pend[-2])
                                    # adaptive depth: drain when the pending
                                    # blocks' score chunks would exceed the
                                    # spsum PSUM banks (deep pipeline for
                                    # small early blocks, shallow for late)
                                    while (sum(nch_of(e) for e in pend) > 6
                                           or len(pend) > 5):
                                        drain_one()
                            emit_S_stats(*pend[-1])
                            while pend:
                                drain_one()
                            drain_at(len(at_pending))

                        # ---- phase 3: final projection, streamed in
                        # quarters (gathered activations load per quarter,
                        # double buffered, overlapping the Wo matmuls) ----
                        with tc.tile_pool(name="fapool", bufs=4) as fapool, \
                             tc.tile_pool(name="fopool", bufs=5) as fopool, \
                             tc.tile_pool(name="fpsum", bufs=8, space="PSUM") as fpsum:
                            NQ = 16
                            QW = L // NQ   # 128 seq cols per slice
                            for q in range(NQ):
                                qsl = slice(q * QW, (q + 1) * QW)
                                atq = fapool.tile([P, HPG, G, QW], F16,
                                                  tag="atq", name=f"atq{q}")
                                for hc in range(HPG):
                                    nc.scalar.dma_start(
                                        atq[:, hc],
                                        ag_outs[hc].rearrange(
                                            "g p l -> p g l")[:, :, qsl])
                                ibs = list(range(q * (IB // NQ),
                                                 (q + 1) * (IB // NQ)))
                                pss = [fpsum.tile([P, FPG], F32, tag=f"f{i}",
                                                  name=f"fps{q}_{i}")
                                       for i in range(len(ibs))]
                                for hc in range(HPG):
                                    for g_idx in range(G):
                                        for i, ib in enumerate(ibs):
                                            nc.tensor.matmul(
                                                pss[i][:],
                                                atq[:, hc, g_idx,
                                                    i * P:(i + 1) * P],
                                                wo_t[:, g_idx * HPG + hc, :],
                                                start=(hc == 0 and g_idx == 0),
                                                stop=(hc == HPG - 1
                                                      and g_idx == G - 1))
                                for i, ib in enumerate(ibs):
                                    o_sb = fopool.tile([P, FPG], F32, tag="fo")
                                    nc.vector.tensor_tensor(
                                        o_sb[:], pss[i][:], bob_t[:], op=OP.add)
                                    nc.sync.dma_start(
                                        out[ib * P:(ib + 1) * P, :], o_sb[:])

    nc.compile()
    return nc


def _split16(x):
    hi = x.astype(np.float16)
    lo = (x - hi.astype(np.float32)).astype(np.float16)
    return hi, lo


def _tile16(x):
    # [D, L] -> [P, NPAN, KC, IPANEL]:  (kc*128+p, ip*512+c) -> [p, ip, kc, c]
    return np.ascontiguousarray(
        x.reshape(KC, P, NPAN, IPANEL).transpose(1, 2, 0, 3))


def _tile8(x):
    # [D, L] -> [P, NPAN, KP, 2, IPANEL]: (kp*256+r*128+p, ip*512+c)
    return np.ascontiguousarray(
        x.reshape(KP, 2, P, NPAN, IPANEL).transpose(2, 3, 0, 1, 4))


def _tilew(w):
    # [D, FPG] -> [P, KC, FPG]
    return np.ascontiguousarray(w.reshape(KC, P, FPG).transpose(1, 0, 2))


def _tilew8(w):
    # [D, FPG] -> [P, KP, 2, FPG]
    return np.ascontiguousarray(w.reshape(KP, 2, P, FPG).transpose(2, 0, 1, 3))


def _prepare_in_maps(q, k, v, Wq, bq, Wk, bk, Wv, bv, Wo, bo):
    mask16 = np.where(
        np.arange(P)[None, :] > np.arange(P)[:, None],
        np.float16(-30000.0), np.float16(0.0)).astype(np.float16)
    ident = np.eye(P, dtype=np.float16)

    f8 = ml_dtypes.float8_e4m3
    xs = {}
    for b in range(B):
        for nm, arr in (("q", q), ("k", k)):
            x = np.ascontiguousarray(arr[b].T, dtype=np.float32)
            hi, lo = _split16(x)
            xs[(nm, b)] = (
                _tile16(hi),
                _tile8(hi.astype(np.float32).astype(f8)),
                _tile8((lo.astype(np.float32) * 2.0 ** 12).astype(f8)),
            )
        xs[("v", b)] = _tile16(
            np.ascontiguousarray(v[b].T, dtype=np.float32).astype(np.float16))

    in_maps = []
    for c in range(8):
        b, g = divmod(c, G)
        F = slice(g * FPG, (g + 1) * FPG)
        rs = np.float32(SCALE ** 0.5)
        wq_h, wq_l = _split16(
            np.ascontiguousarray(Wq[F, :].T, dtype=np.float32) * rs)
        wk_h, wk_l = _split16(
            np.ascontiguousarray(Wk[F, :].T, dtype=np.float32) * rs)
        w8 = {}
        for nm, (wh_, wl_) in (("q", (wq_h, wq_l)), ("k", (wk_h, wk_l))):
            w8[nm] = (
                _tilew8((wh_.astype(np.float32) * 2.0 ** 5).astype(f8)),
                _tilew8((wl_.astype(np.float32) * 2.0 ** 17).astype(f8)),
            )
        in_maps.append({
            "xqh": xs[("q", b)][0],
            "xq8h": xs[("q", b)][1], "xq8l": xs[("q", b)][2],
            "xkh": xs[("k", b)][0],
            "xk8h": xs[("k", b)][1], "xk8l": xs[("k", b)][2],
            "xvh": xs[("v", b)],
            "wqh": _tilew(wq_h), "wq8h": w8["q"][0], "wq8l": w8["q"][1],
            "wkh": _tilew(wk_h), "wk8h": w8["k"][0], "wk8l": w8["k"][1],
            "wvh": _tilew(np.ascontiguousarray(Wv[F, :].T).astype(np.float16)),
            "woT": _tilew(np.ascontiguousarray(Wo[F, :].T).astype(np.float16)),
            "bq": np.ascontiguousarray(
                (bq[F] * rs).astype(np.float32).reshape(HPG, P).T),
            "bk": np.ascontiguousarray(
                (bk[F] * rs).astype(np.float32).reshape(HPG, P).T),
            "bvb": np.broadcast_to(bv[F][None, :], (P, FPG)).astype(np.float32),
            "bob": np.broadcast_to(bo[F][None, :], (P, FPG)).astype(np.float32),
            "maskh": mask16,
            "identd": ident,
        })
    return in_maps


def kernel(**inputs) -> np.ndarray:
    global _COMPILED
    from concourse.bass_utils import run_bass_kernel_spmd

    if _COMPILED is None:
        _COMPILED = _build()
    nc = _COMPILED

    in_maps = _prepare_in_maps(**inputs)
    res = run_bass_kernel_spmd(nc, in_maps, list(range(8)))

    outp = np.empty((B, L, D), dtype=np.float32)
    for c in range(8):
        b, g = divmod(c, G)
        outp[b, :, g * FPG:(g + 1) * FPG] = res.results[c]["out"]
    return outp


if __name__ == "__main__":
    rng = np.random.default_rng(1)
    ins = {
        "q": rng.standard_normal((B, L, D), dtype=np.float32),
        "k": rng.standard_normal((B, L, D), dtype=np.float32),
        "v": rng.standard_normal((B, L, D), dtype=np.float32),
        "Wq": rng.standard_normal((D, D), dtype=np.float32) * 0.02,
        "bq": rng.standard_normal(D).astype(np.float32) * 0.02,
        "Wk": rng.standard_normal((D, D), dtype=np.float32) * 0.02,
        "bk": rng.standard_normal(D).astype(np.float32) * 0.02,
        "Wv": rng.standard_normal((D, D), dtype=np.float32) * 0.02,
        "bv": rng.standard_normal(D).astype(np.float32) * 0.02,
        "Wo": rng.standard_normal((D, D), dtype=np.float32) * 0.02,
        "bo": rng.standard_normal(D).astype(np.float32) * 0.02,
    }
    o = kernel(**ins)
    print("kernel ran, out shape", o.shape)



# revision 41
# speedup vs baseline: 1.0455x; 1.0455x over previous
"""Trainium2 Bass kernel for causal multi-head attention (B=2, L=2048, D=2048,
H=16 heads, DH=128), sharded over 8 NeuronCores.

Sharding: core c handles batch b=c//4 and head-group g=c%4 (4 heads = 512
features). The only cross-core communication is a per-head, per-L-quarter
fp16 AllGather of attention outputs within each 4-core batch group.

v4 design (fp32r + panel-major interleave + overlapped output projection):
- Q/K projections and the score matmuls run in float32r: the PE processes
  f32r at 1 col/cycle when the moving dim is >= 256 (same speed as fp16),
  while storage rounds to ~14 mantissa bits — accurate enough (out rel err
  ~1e-2 vs the 2e-2 budget) to drop v2's compensated fp16+fp8 DoubleRow
  correction scheme entirely, with its plane-splitting evacuations.
- Attention runs PANEL-MAJOR: groups g=0..7 process seq blocks {2g, 2g+1}
  of all 4 heads. K/Q projection subtasks (one head x one 256-seq panel, 16
  matmuls + bias evac) interleave one-per-block into group g for panel g+1,
  so the PE always has dense matmul work while the softmax chain (DVE max,
  ACT exp, Pool/DVE pscale, DMA transpose) drains. qth lives as 2 rotating
  panel tiles; kth accumulates in a full 4MB f32r tile. V projection is its
  own DMA-bound front phase with the K0 subtasks injected into its tail.
- Block pipeline: S(n) | stats(n-1: DVE chunk-max + ACT exp w/ accum row
  sums) | scale+transpose(n-2) | AV(n-4). Stats/scale of older blocks are
  emitted BEFORE S(n): pool WAR tracking is emission-ordered, so S(n) must
  see the exp instructions that free the score banks it reuses.
- Output projection OVERLAPS the attention tail: gathers run per L-quarter
  as soon as all heads finish a quarter (spread 2 copies/slot so they never
  monopolize the sync queue ahead of transposes); after group 6 the wq/wk/x
  pools (right-side SBUF stack) close and the Wo pools open in their place,
  and Wo half-slices fill group 7 + the drain, interleaved with the last
  AV blocks. Wo reads gathered activations in 256-seq slices so every
  descriptor is 512B (full DMA rate).
- Engine split: DVE = maxes + stats + o-evac; ACT = exp + proj evacs; Pool
  = pscales (even chunks early, all chunks late) + x-load DMAs (SWDGE) +
  out writes; PE adds the causal mask via the id^T @ mask trick. x loads
  ride the Pool queue as 0.53MB quarter-tiles with ~2-panel-deep rotation
  (bufs=8) so their WAR waits rarely head-of-line-block the pscales.
- PSUM: 5 score banks + 2 O^T banks + 1 proj bank (each O^T accumulator
  must own a bank; score chunks use widths >=256, e.g. 384+256 instead of
  512+128, to stay at the f32r fast rate).
- Known dead ends (all measured slower in TimelineSim): pscales all on DVE
  (blocks the max chain) or x loads on sync/scalar (blocks transposes/exps
  — every queue hosts something latency-critical, Pool loses the least);
  K-evacs on DVE; panel tasks emitted before the block push; sliding half
  of each panel's subtasks one group later (intra-group deadlines too
  tight); ppsum=2/spsum=4 (the 5th score bank wins); deeper pend without
  more pt buffers (SBUF-capped at ~205KB/partition of 208).
"""
import sys

sys.path.insert(0, "/opt/trn_rl_repo")

import numpy as np

B, L, D, H = 2, 2048, 2048, 16
DH = D // H          # 128
G = 4                # head-groups (tensor-parallel degree per batch)
HPG = H // G         # heads per group = 4
FPG = HPG * DH       # features per group = 512
P = 128
SCALE = float(DH) ** 0.5
KC = D // P          # 16 contraction chunks
IB = L // P          # 16 seq blocks of 128
PAN = 256            # K/Q/V projection seq panel
NPAN = L // PAN      # 8
NGRP = IB // 2       # 8 block groups (2 blocks x 4 heads each)

_COMPILED = None


def _chunk_widths(nj):
    """Split nj into chunks of <=512 with every chunk >=256 when possible
    (f32r matmuls run 4x slower below a 256-wide moving dim)."""
    if nj <= 512:
        return [nj]
    nch = (nj + 511) // 512
    rem = nj - 512 * (nch - 1)
    w = [512] * (nch - 1) + [rem]
    if rem < 256:
        w[-2:] = [512 + rem - 256, 256]
    return w


def _build(variant="main"):
    import concourse.bacc as bacc
    import concourse.tile as tile
    from concourse import mybir
    from contextlib import ExitStack

    F32 = mybir.dt.float32
    F32R = mybir.dt.float32r
    F16 = mybir.dt.float16
    AX = mybir.AxisListType
    OP = mybir.AluOpType
    ACTF = mybir.ActivationFunctionType

    nc = bacc.Bacc("TRN2", target_bir_lowering=False, debug=False, num_devices=8)

    # ---- DRAM I/O (x/w pre-arranged host-side in tile order) ----
    xq = nc.dram_tensor("xq", [P, NPAN, KC, PAN], F32R, kind="ExternalInput")
    xk = nc.dram_tensor("xk", [P, NPAN, KC, PAN], F32R, kind="ExternalInput")
    xv = nc.dram_tensor("xv", [P, NPAN, KC, PAN], F16, kind="ExternalInput")
    wq = nc.dram_tensor("wq", [P, KC, FPG], F32R, kind="ExternalInput")
    wk = nc.dram_tensor("wk", [P, KC, FPG], F32R, kind="ExternalInput")
    wv = nc.dram_tensor("wv", [P, KC, FPG], F16, kind="ExternalInput")
    woT = nc.dram_tensor("woT", [P, KC, FPG], F16, kind="ExternalInput")
    bq = nc.dram_tensor("bq", [P, HPG], F32, kind="ExternalInput")
    bk = nc.dram_tensor("bk", [P, HPG], F32, kind="ExternalInput")
    bvb = nc.dram_tensor("bvb", [P, FPG], F32, kind="ExternalInput")
    bob = nc.dram_tensor("bob", [P, FPG], F32, kind="ExternalInput")
    maskh = nc.dram_tensor("maskh", [P, P], F16, kind="ExternalInput")
    identd = nc.dram_tensor("identd", [P, P], F16, kind="ExternalInput")
    out = nc.dram_tensor("out", [L, FPG], F32, kind="ExternalOutput")
    if variant == "timing":
        chain = nc.dram_tensor("chain", [1, 8], F32, kind="ExternalInput")
        dummy = nc.dram_tensor("chaino", [1, 8], F32, kind="ExternalOutput")

    with tile.TileContext(nc) as tc:
        with ExitStack() as ctx:
            consts = ctx.enter_context(tc.tile_pool(name="consts", bufs=1))

            maskh_t = consts.tile([P, P], F16)
            id_t = consts.tile([P, P], F16)
            bq_t = consts.tile([P, HPG], F32)
            bk_t = consts.tile([P, HPG], F32)
            bvb_t = consts.tile([P, FPG], F32)
            nc.scalar.dma_start(bvb_t[:], bvb[:])
            nc.scalar.dma_start(bq_t[:], bq[:])
            nc.scalar.dma_start(bk_t[:], bk[:])
            bob_t = consts.tile([P, FPG], F32)
            if variant == "timing":
                ch_t = consts.tile([1, 8], F32)
                nc.sync.dma_start(ch_t[:], chain[:])
                nc.sync.dma_start(dummy[:], ch_t[:])

            NREP = {"x4": 4, "x2": 2, "x2nc": 2}.get(variant, 1)
            for _rep in range(NREP):
                ag_outs = []
                with tc.tile_pool(name="qkv", bufs=1) as qkv:
                    kth = qkv.tile([P, HPG, L], F32R)   # (dh, head, j)
                    vt = qkv.tile([P, IB, FPG], F16)    # (j%128, blk, feat)

                    with tc.tile_pool(name="qpan", bufs=2) as qpanp, \
                         tc.tile_pool(name="dramio", bufs=1,
                                      space="DRAM") as dramio:
                        ag_ins = []
                        for hh in range(HPG):
                            # per-L-quarter tiles: collectives need
                            # contiguous access patterns
                            ag_outs.append([dramio.tile(
                                [G, P, 512], F16, tag=f"agout{hh}_{lq}",
                                name=f"agout{hh}_{lq}") for lq in range(4)])
                            if variant not in ("nocoll", "x2nc"):
                                ag_ins.append([dramio.tile(
                                    [P, 512], F16, tag=f"agin{hh}_{lq}",
                                    name=f"agin{hh}_{lq}")
                                    for lq in range(4)])

                        def emit_gather_lq(lq, part=None):
                            # part: emit only 2 of the 16 nocoll copies so
                            # a gather never monopolizes the sync queue
                            # ahead of latency-critical transposes
                            lsl = slice(lq * 512, (lq + 1) * 512)
                            if variant in ("nocoll", "x2nc"):
                                pairs = [(hh, gg) for hh in range(HPG)
                                         for gg in range(G)]
                                if part is not None:
                                    pairs = pairs[2 * part:2 * part + 2]
                                for hh, gg in pairs:
                                    nc.sync.dma_start(
                                        ag_outs[hh][lq][gg][:],
                                        ots[hh][:, lsl])
                            else:
                                if part not in (None, 0):
                                    return
                                for hh in range(HPG):
                                    nc.scalar.dma_start(
                                        ag_ins[hh][lq][:],
                                        ots[hh][:, lsl])
                                    nc.gpsimd.collective_compute(
                                        "AllGather", OP.bypass,
                                        replica_groups=[[0, 1, 2, 3],
                                                        [4, 5, 6, 7]],
                                        ins=[ag_ins[hh][lq][:].opt()],
                                        outs=[ag_outs[hh][lq][:].opt()])

                        proj_ctx = ExitStack()
                        wqk = proj_ctx.enter_context(
                            tc.tile_pool(name="wqk", bufs=1, side="right"))
                        xqkp = proj_ctx.enter_context(
                            tc.tile_pool(name="xqk", bufs=8, side="right"))
                        ppsum = proj_ctx.enter_context(
                            tc.tile_pool(name="ppsum", bufs=1,
                                         space="PSUM", side="right"))
                        wq_t = wqk.tile([P, KC, FPG], F32R)
                        wk_t = wqk.tile([P, KC, FPG], F32R)

                        # ---- phase 1: V projection (fp16) ----
                        with tc.tile_pool(name="wvp", bufs=1,
                                          side="right") as wvp, \
                             tc.tile_pool(name="xvp", bufs=2,
                                          side="right") as xvp, \
                             tc.tile_pool(name="vpsum", bufs=2,
                                          space="PSUM",
                                          side="right") as vpsum:
                            wv_t = wvp.tile([P, KC, FPG], F16)
                            for ip in range(NPAN):
                                xv_t = xvp.tile([P, KC, PAN], F16, tag="xv")
                                if ip == 0:
                                    # interleave w/x quarters so the first
                                    # matmul's operands land first
                                    for xc in range(4):
                                        ksl = slice(xc * 4, xc * 4 + 4)
                                        nc.sync.dma_start(xv_t[:, ksl],
                                                          xv[:, ip, ksl])
                                        nc.sync.dma_start(
                                            wv_t[:, ksl], wv[:, ksl])
                                else:
                                    nc.sync.dma_start(xv_t[:], xv[:, ip])
                                if 1 <= ip <= 4:
                                    # prefetch K/Q weights on the scalar
                                    # queue, quartered so xv panels interleave
                                    # at the (serial) DMA transfer resource
                                    ksl = slice((ip - 1) * 4, ip * 4)
                                    nc.scalar.dma_start(wk_t[:, ksl],
                                                        wk[:, ksl])
                                if 3 <= ip <= 6:
                                    ksl = slice((ip - 3) * 4, (ip - 2) * 4)
                                    nc.scalar.dma_start(wq_t[:, ksl],
                                                        wq[:, ksl])
                                if ip == 5 and _rep == 0:
                                    nc.scalar.dma_start(maskh_t[:], maskh[:])
                                    nc.scalar.dma_start(id_t[:], identd[:])
                                    nc.scalar.dma_start(bob_t[:], bob[:])
                                for sub in range(PAN // P):
                                    ib = ip * (PAN // P) + sub
                                    ps = vpsum.tile([P, FPG], F32, tag="pv")
                                    for kc in range(KC):
                                        nc.tensor.matmul(
                                            ps[:],
                                            xv_t[:, kc, sub * P:(sub + 1) * P],
                                            wv_t[:, kc, :],
                                            start=(kc == 0),
                                            stop=(kc == KC - 1))
                                    nc.vector.tensor_tensor(
                                        vt[:, ib, :], ps[:], bvb_t[:],
                                        op=OP.add)

                        # ---- K/Q projection panel subtasks ----
                        xtiles = {}
                        qpan_tiles = {}

                        def issue_x(kind, p):
                            """Issue the x-panel load for (kind, p) as 4
                            quarter tiles (0.53MB each). Quarter-granular
                            pool rotation keeps the WAR semaphore of each
                            DMA ~2 panels back, so a blocked transfer never
                            head-of-line-blocks the Pool queue for long."""
                            src = xq if kind == "q" else xk
                            qts = []
                            for qc in range(4):
                                ksl = slice(qc * 4, (qc + 1) * 4)
                                qt = xqkp.tile([P, 4, PAN], F32R, tag="xh",
                                               name=f"x{kind}{p}_{qc}")
                                nc.gpsimd.dma_start(qt[:], src[:, p, ksl])
                                qts.append(qt)
                            xtiles[(kind, p)] = qts

                        def emit_panel(kind, p, h):
                            qts = xtiles[(kind, p)]
                            if kind == "q" and p not in qpan_tiles:
                                qpan_tiles[p] = qpanp.tile(
                                    [P, HPG, PAN], F32R, tag="qp",
                                    name=f"qp{p}")
                            w_t = wq_t if kind == "q" else wk_t
                            bias_t = bq_t if kind == "q" else bk_t
                            fsl = slice(h * P, (h + 1) * P)
                            ps = ppsum.tile([P, 512], F32, tag="pp")
                            for kc in range(KC):
                                nc.tensor.matmul(
                                    ps[:, :PAN], w_t[:, kc, fsl],
                                    qts[kc // 4][:, kc % 4, :],
                                    start=(kc == 0), stop=(kc == KC - 1))
                            if kind == "q":
                                dst = qpan_tiles[p][:, h, :]
                            else:
                                dst = kth[:, h, p * PAN:(p + 1) * PAN]
                            nc.scalar.activation(
                                dst, ps[:, :PAN], ACTF.Identity,
                                bias=bias_t[:, h:h + 1], scale=1.0)

                        # prologue: panel 0 of K and Q; x for panels 0-1
                        issue_x("k", 0)
                        issue_x("q", 0)
                        for h in range(HPG):
                            emit_panel("k", 0, h)
                        issue_x("k", 1)
                        for h in range(HPG):
                            emit_panel("q", 0, h)
                        issue_x("q", 1)
                        # panel-2 x rides during group 0 (slots 3 and 7)

                        # ---- phase 2: attention, panel-major ----
                        with tc.tile_pool(name="otpool", bufs=1) as otpool, \
                             tc.tile_pool(name="ppool", bufs=4) as ppool, \
                             tc.tile_pool(name="ptpool", bufs=4) as ptpool, \
                             tc.tile_pool(name="stats", bufs=8) as stats, \
                             tc.tile_pool(name="spsum", bufs=5,
                                          space="PSUM") as spsum, \
                             tc.tile_pool(name="opsum", bufs=2,
                                          space="PSUM") as opsum:
                            ots = [otpool.tile([P, L], F16,
                                               name=f"ot{hh}")
                                   for hh in range(HPG)]

                            def emit_S(h, ib):
                                nj = (ib + 1) * P
                                widths = _chunk_widths(nj)
                                isl = slice(ib * P, (ib + 1) * P)
                                mpart = stats.tile([P, 4], F32, tag="mp",
                                                   name=f"mp{h}_{ib}")
                                lpart = stats.tile([P, 4], F32, tag="lp",
                                                   name=f"lp{h}_{ib}")
                                qt = qpan_tiles[ib // 2]
                                qsl = qt[:, h,
                                         (ib % 2) * P:(ib % 2 + 1) * P]
                                chunks = []
                                off = 0
                                for jc, w in enumerate(widths):
                                    diag = jc == len(widths) - 1
                                    ps = spsum.tile([P, 512], F32, tag="s",
                                                    name=f"sps{h}_{ib}_{jc}")
                                    nc.tensor.matmul(
                                        ps[:, :w], qsl,
                                        kth[:, h, off:off + w],
                                        start=True, stop=not diag)
                                    if diag:
                                        # causal mask on the diagonal block,
                                        # accumulated on the PE: += I.T @ mask
                                        nc.tensor.matmul(
                                            ps[:, w - P:w], id_t[:],
                                            maskh_t[:],
                                            start=False, stop=True)
                                    chunks.append((ps, w, off, jc))
                                    off += w
                                return mpart, lpart, chunks

                            def emit_S_stats(e):
                                # scores arrive pre-scaled (host folds
                                # sqrt(scale) into Wq/Wk): negated chunk
                                # max IS the exp bias
                                mpart, lpart = e["mpart"], e["lpart"]
                                p_sb = ppool.tile([P, L], F16, tag="p",
                                                  name=f"p{e['h']}_{e['ib']}")
                                e["p_sb"] = p_sb
                                for ps, w, off, jc in e["chunks"]:
                                    nc.vector.reduce_max(
                                        mpart[:, jc:jc + 1], ps[:, :w],
                                        axis=AX.X, negate=True)
                                    nc.scalar.activation(
                                        p_sb[:, off:off + w], ps[:, :w],
                                        ACTF.Exp, bias=mpart[:, jc:jc + 1],
                                        scale=1.0,
                                        accum_out=lpart[:, jc:jc + 1])

                            def emit_scale_tr(e):
                                h, ib, p_sb, mpart, lpart, chunks = (
                                    e["h"], e["ib"], e["p_sb"], e["mpart"],
                                    e["lpart"], e["chunks"])
                                nch = len(chunks)
                                rmin = stats.tile([P, 1], F32, tag="nm",
                                                  name=f"nm{h}_{ib}")
                                nc.vector.tensor_reduce(
                                    rmin[:], mpart[:, :nch], axis=AX.X,
                                    op=OP.min)
                                # per-chunk correction c = exp(m_jc - m)
                                cfac = stats.tile([P, 4], F32, tag="cf",
                                                  name=f"cf{h}_{ib}")
                                nc.scalar.activation(
                                    cfac[:, :nch], mpart[:, :nch],
                                    ACTF.Exp, bias=rmin[:], scale=-1.0)
                                lw = stats.tile([P, 4], F32, tag="lw",
                                                name=f"lw{h}_{ib}")
                                nc.vector.tensor_tensor(
                                    lw[:, :nch], cfac[:, :nch],
                                    lpart[:, :nch], op=OP.mult)
                                lsum = stats.tile([P, 1], F32, tag="ls",
                                                  name=f"ls{h}_{ib}")
                                nc.vector.reduce_sum(lsum[:], lw[:, :nch],
                                                     axis=AX.X)
                                rinv = stats.tile([P, 1], F32, tag="ri",
                                                  name=f"ri{h}_{ib}")
                                nc.vector.reciprocal(rinv[:], lsum[:])
                                # P_jc *= c_jc * rinv, chunks alternating
                                # Pool/DVE so the stage's serial latency
                                # halves (Pool is SBUF-only; DVE runs 4x on
                                # packed fp16)
                                for ps, w, off, jc in chunks:
                                    # late groups: x-load DMAs are done, the
                                    # Pool queue is empty -> all pscales to
                                    # Pool, freeing DVE for the maxes
                                    if ib >= 10:
                                        eng = nc.gpsimd
                                    else:
                                        eng = nc.gpsimd if jc % 2 == 0 \
                                            else nc.vector
                                    eng.tensor_scalar(
                                        p_sb[:, off:off + w],
                                        p_sb[:, off:off + w],
                                        cfac[:, jc:jc + 1], rinv[:],
                                        op0=OP.mult, op1=OP.mult)
                                pt_sb = ptpool.tile([P, IB, P], F16,
                                                    tag="ptsb",
                                                    name=f"ptsb{h}_{ib}")
                                nblk = ib + 1
                                nc.sync.dma_start_transpose(
                                    pt_sb[:, :nblk, :], p_sb[:, :nblk * P])
                                e["pt_sb"] = pt_sb

                            def emit_av(e):
                                h, ib, pt_sb = e["h"], e["ib"], e["pt_sb"]
                                isl = slice(ib * P, (ib + 1) * P)
                                o_ps = opsum.tile([P, P], F32, tag="o",
                                                  name=f"o{h}_{ib}")
                                for jb in range(ib + 1):
                                    nc.tensor.matmul(
                                        o_ps[:],
                                        vt[:, jb, h * P:(h + 1) * P],
                                        pt_sb[:, jb, :],
                                        start=(jb == 0), stop=(jb == ib))
                                nc.vector.tensor_copy(
                                    ots[h][:, isl], o_ps[:])

                            # 4-stage pipeline: S(n) | stats(n-1) |
                            # scale+transpose(n-2) | AV(n-4) — each block's
                            # transpose gets ~2 block-iterations of wall
                            # time before its AV matmuls hit the PE
                            pend = []

                            def push_block(h, ib):
                                # stats/scale of older blocks FIRST: the
                                # spsum/ppool rotations must see those
                                # reader instructions before S(n) reuses
                                # their buffers (pool WAR is emission-order)
                                if len(pend) >= 1:
                                    emit_S_stats(pend[-1])
                                if len(pend) >= 2:
                                    emit_scale_tr(pend[-2])
                                mpart, lpart, chunks = emit_S(h, ib)
                                pend.append(dict(
                                    h=h, ib=ib, mpart=mpart,
                                    lpart=lpart, chunks=chunks))
                                while len(pend) > 4:
                                    emit_av(pend.pop(0))

                            wo_state = {}

                            def open_wo():
                                # proj pools are done (last panel task was
                                # emitted in group 6): close them LIFO and
                                # open the output-projection pools in their
                                # space, so Wo slices can fill the PE-idle
                                # attention tail
                                proj_ctx.close()
                                c = ExitStack()
                                wo_state["ctx"] = c
                                ph3 = c.enter_context(
                                    tc.tile_pool(name="ph3", bufs=1,
                                                 side="right"))
                                wo_state["fap"] = c.enter_context(
                                    tc.tile_pool(name="fapool", bufs=2,
                                                 side="right"))
                                wo_state["fop"] = c.enter_context(
                                    tc.tile_pool(name="fopool", bufs=2,
                                                 side="right"))
                                wo_state["fps"] = c.enter_context(
                                    tc.tile_pool(name="fpsum", bufs=1,
                                                 space="PSUM",
                                                 side="right"))
                                wo_t = ph3.tile([P, KC, FPG], F16,
                                                name=f"wo{_rep}")
                                for qc in range(4):
                                    ksl = slice(qc * 4, (qc + 1) * 4)
                                    nc.sync.dma_start(wo_t[:, ksl],
                                                      woT[:, ksl])
                                wo_state["wo_t"] = wo_t

                            def emit_wo_half(q, i):
                                qsl = slice(q * 256, (q + 1) * 256)
                                wo_t = wo_state["wo_t"]
                                if i == 0:
                                    atq = wo_state["fap"].tile(
                                        [P, HPG, G, 256], F16, tag="atq",
                                        name=f"atq{q}")
                                    eng = nc.sync if q < 2 else nc.scalar
                                    csl = slice((q % 2) * 256,
                                                (q % 2) * 256 + 256)
                                    for hc in range(HPG):
                                        eng.dma_start(
                                            atq[:, hc],
                                            ag_outs[hc][q // 2].rearrange(
                                                "g p l -> p g l")[:, :,
                                                                  csl])
                                    wo_state["atq"] = atq
                                atq = wo_state["atq"]
                                ps = wo_state["fps"].tile(
                                    [P, FPG], F32, tag="f",
                                    name=f"fps{q}_{i}")
                                for hc in range(HPG):
                                    for g_idx in range(G):
                                        nc.tensor.matmul(
                                            ps[:],
                                            atq[:, hc, g_idx,
                                                i * P:(i + 1) * P],
                                            wo_t[:, g_idx * HPG + hc, :],
                                            start=(hc == 0 and g_idx == 0),
                                            stop=(hc == HPG - 1
                                                  and g_idx == G - 1))
                                o_sb = wo_state["fop"].tile(
                                    [P, FPG], F32, tag="fo")
                                nc.vector.tensor_tensor(
                                    o_sb[:], ps[:], bob_t[:], op=OP.add)
                                nc.gpsimd.dma_start(
                                    out[(2 * q + i) * P:
                                        (2 * q + i + 1) * P, :],
                                    o_sb[:])

                            for g in range(NGRP):
                                p = g + 1
                                tasks = []
                                if p < NPAN:
                                    tasks = ([("k", p, hh)
                                              for hh in range(HPG)] +
                                             [("q", p, hh)
                                              for hh in range(HPG)])
                                ti = 0
                                for h in range(HPG):
                                    for sub in range(2):
                                        ib = 2 * g + sub
                                        push_block(h, ib)
                                        if ti < len(tasks):
                                            emit_panel(*tasks[ti])
                                            ti += 1
                                        if h == 1 and sub == 1 \
                                                and g + 2 < NPAN:
                                            issue_x("k", g + 2)
                                        if h == 3 and sub == 1 \
                                                and g + 2 < NPAN:
                                            issue_x("q", g + 2)
                                        slot = 2 * h + sub
                                        if g == 3:
                                            emit_gather_lq(0, slot)
                                        if g == 5:
                                            emit_gather_lq(1, slot)
                                        if g == 6 and h >= 2:
                                            emit_gather_lq(2, slot - 4)
                                        if g == 7 and h < 2:
                                            emit_gather_lq(2, slot + 4)
                                        if g == 7 and h == 1 and sub == 0:
                                            emit_wo_half(0, 0)
                                        if g == 7 and h == 2 and sub == 0:
                                            emit_wo_half(0, 1)
                                        if g == 7 and h == 3 and sub == 0:
                                            emit_wo_half(1, 0)
                                        if g == 7 and h == 3 and sub == 1:
                                            emit_wo_half(1, 1)
                                if g == 6:
                                    open_wo()
                            emit_S_stats(pend[-1])
                            emit_scale_tr(pend[-2])
                            emit_scale_tr(pend[-1])
                            halves = [(q, i) for q in (2, 3, 4, 5)
                                      for i in (0, 1)]
                            hi = 0
                            while pend:
                                emit_av(pend.pop(0))
                                for _ in range(2):
                                    if hi < len(halves):
                                        emit_wo_half(*halves[hi])
                                        hi += 1
                            while hi < len(halves):
                                emit_wo_half(*halves[hi])
                                hi += 1
                            emit_gather_lq(3)
                            for q in (6, 7):
                                for i in (0, 1):
                                    emit_wo_half(q, i)
                            wo_state["ctx"].close()

    nc.compile()
    return nc


def _tilex(x, dtype):
    # [D, L] -> [P, NPAN, KC, PAN]: (kc*128+p, pl*PAN+c) -> [p, pl, kc, c]
    return np.ascontiguousarray(
        x.reshape(KC, P, NPAN, PAN).transpose(1, 2, 0, 3).astype(dtype))


def _tilew(w, dtype):
    # [D, FPG] -> [P, KC, FPG]
    return np.ascontiguousarray(
        w.reshape(KC, P, FPG).transpose(1, 0, 2).astype(dtype))


def _prepare_in_maps(q, k, v, Wq, bq, Wk, bk, Wv, bv, Wo, bo):
    mask16 = np.where(
        np.arange(P)[None, :] > np.arange(P)[:, None],
        np.float16(-30000.0), np.float16(0.0)).astype(np.float16)
    ident = np.eye(P, dtype=np.float16)

    xs = {}
    for b in range(B):
        for nm, arr in (("q", q), ("k", k)):
            xs[(nm, b)] = _tilex(
                np.ascontiguousarray(arr[b].T, dtype=np.float32), np.float32)
        xs[("v", b)] = _tilex(
            np.ascontiguousarray(v[b].T, dtype=np.float32), np.float16)

    rs = np.float32(SCALE ** 0.5)
    in_maps = []
    for c in range(8):
        b, g = divmod(c, G)
        F = slice(g * FPG, (g + 1) * FPG)
        in_maps.append({
            "xq": xs[("q", b)],
            "xk": xs[("k", b)],
            "xv": xs[("v", b)],
            "wq": _tilew(
                np.ascontiguousarray(Wq[F, :].T, dtype=np.float32) * rs,
                np.float32),
            "wk": _tilew(
                np.ascontiguousarray(Wk[F, :].T, dtype=np.float32) * rs,
                np.float32),
            "wv": _tilew(
                np.ascontiguousarray(Wv[F, :].T, dtype=np.float32),
                np.float16),
            "woT": _tilew(
                np.ascontiguousarray(Wo[F, :].T, dtype=np.float32),
                np.float16),
            "bq": np.ascontiguousarray(
                (bq[F] * rs).astype(np.float32).reshape(HPG, P).T),
            "bk": np.ascontiguousarray(
                (bk[F] * rs).astype(np.float32).reshape(HPG, P).T),
            "bvb": np.broadcast_to(bv[F][None, :], (P, FPG)).astype(
                np.float32),
            "bob": np.broadcast_to(bo[F][None, :], (P, FPG)).astype(
                np.float32),
            "maskh": mask16,
            "identd": ident,
        })
    return in_maps


def kernel(**inputs) -> np.ndarray:
    global _COMPILED
    from concourse.bass_utils import run_bass_kernel_spmd

    if _COMPILED is None:
        _COMPILED = _build()
    nc = _COMPILED

    in_maps = _prepare_in_maps(**inputs)
    res = run_bass_kernel_spmd(nc, in_maps, list(range(8)))

    outp = np.empty((B, L, D), dtype=np.float32)
    for c in range(8):
        b, g = divmod(c, G)
        outp[b, :, g * FPG:(g + 1) * FPG] = res.results[c]["out"]
    return outp


if __name__ == "__main__":
    rng = np.random.default_rng(1)
    ins = {
        "q": rng.standard_normal((B, L, D), dtype=np.float32),
        "k": rng.standard_normal((B, L, D), dtype=np.float32),
        "v": rng.standard_normal((B, L, D), dtype=np.float32),
        "Wq": rng.standard_normal((D, D), dtype=np.float32) * 0.02,
        "bq": rng.standard_normal(D).astype(np.float32) * 0.02,
        "Wk": rng.standard_normal((D, D), dtype=np.float32) * 0.02,
        "bk": rng.standard_normal(D).astype(np.float32) * 0.02,
        "Wv": rng.standard_normal((D, D), dtype=np.float32) * 0.02,
        "bv": rng.standard_normal(D).astype(np.float32) * 0.02,
        "Wo": rng.standard_normal((D, D), dtype=np.float32) * 0.02,
        "bo": rng.standard_normal(D).astype(np.float32) * 0.02,
    }
    o = kernel(**ins)
    print("kernel ran, out shape", o.shape)


# revision 42
# speedup vs baseline: 1.0510x; 1.0053x over previous
"""Trainium2 Bass kernel for causal multi-head attention (B=2, L=2048, D=2048,
H=16 heads, DH=128), sharded over 8 NeuronCores.

Sharding: core c handles batch b=c//4 and head-group g=c%4 (4 heads = 512
features). The only cross-core communication is a per-head fp16 AllGather of
attention outputs within each 4-core batch group.

v3 design (fp32r + panel-major interleave):
- Q/K projections and the score matmuls run in float32r: the PE processes
  f32r at 1 col/cycle when the moving dim is >= 256 (same as fp16), while
  storage rounds to ~14 mantissa bits — accurate enough to drop the whole
  compensated fp16+fp8 DoubleRow correction scheme of v2 (S abs err ~0.02 vs
  the 0.109 budget).
- Attention runs PANEL-MAJOR: groups g=0..7 process seq blocks {2g, 2g+1} of
  all 4 heads. K/Q projection panel subtasks (one head x one 256-seq panel,
  16 matmuls) interleave one-per-block into group g for panel g+1, so the PE
  always has dense matmul work while the softmax chain (DVE max, ACT exp,
  Pool pscale, DMA transpose) drains. qth lives as 2 rotating 256-seq panel
  tiles; kth accumulates in a full 4MB f32r tile.
- Engine split: DVE = chunk maxes + stats; ACT = exp (accum_out row sums) +
  proj/o evacuations; Pool = pscales (Pool cannot touch PSUM); PE adds the
  causal mask via the id^T @ mask accumulation trick.
- DMA queues: sync = V-phase loads + P^T transposes + nocoll gather copies;
  scalar = consts + wq/wk prefetch + ag bounce + wo + atq; vector = xq/xk
  half-tile streams + out writes. Keeps the latency-critical transposes from
  queuing behind bulk loads.
- Wo phase reads gathered activations in 256-seq slices so every descriptor
  is 512B (full DMA rate; 128-seq slices pay the sub-512B 2x penalty).
- PSUM: 2 proj banks + 4 score banks + 2 O^T banks (each O^T accumulator
  must own a bank: concurrent groups sharing a bank corrupt via bank-granular
  has_written clears). Score chunks use widths >=256 (e.g. 384+256 instead
  of 512+128) to stay at the f32r fast rate.
"""
import sys

sys.path.insert(0, "/opt/trn_rl_repo")

import numpy as np

B, L, D, H = 2, 2048, 2048, 16
DH = D // H          # 128
G = 4                # head-groups (tensor-parallel degree per batch)
HPG = H // G         # heads per group = 4
FPG = HPG * DH       # features per group = 512
P = 128
SCALE = float(DH) ** 0.5
KC = D // P          # 16 contraction chunks
IB = L // P          # 16 seq blocks of 128
PAN = 256            # K/Q/V projection seq panel
NPAN = L // PAN      # 8
NGRP = IB // 2       # 8 block groups (2 blocks x 4 heads each)

_COMPILED = None


def _chunk_widths(nj):
    """Split nj into chunks of <=512 with every chunk >=256 when possible
    (f32r matmuls run 4x slower below a 256-wide moving dim)."""
    if nj <= 512:
        return [nj]
    nch = (nj + 511) // 512
    rem = nj - 512 * (nch - 1)
    w = [512] * (nch - 1) + [rem]
    if rem < 256:
        w[-2:] = [512 + rem - 256, 256]
    return w


def _build(variant="main"):
    import concourse.bacc as bacc
    import concourse.tile as tile
    from concourse import mybir
    from contextlib import ExitStack

    F32 = mybir.dt.float32
    F32R = mybir.dt.float32r
    F16 = mybir.dt.float16
    AX = mybir.AxisListType
    OP = mybir.AluOpType
    ACTF = mybir.ActivationFunctionType

    nc = bacc.Bacc("TRN2", target_bir_lowering=False, debug=False, num_devices=8)

    # ---- DRAM I/O (x/w pre-arranged host-side in tile order) ----
    xq = nc.dram_tensor("xq", [P, NPAN, KC, PAN], F32R, kind="ExternalInput")
    xk = nc.dram_tensor("xk", [P, NPAN, KC, PAN], F32R, kind="ExternalInput")
    xv = nc.dram_tensor("xv", [P, NPAN, KC, PAN], F16, kind="ExternalInput")
    wq = nc.dram_tensor("wq", [P, KC, FPG], F32R, kind="ExternalInput")
    wk = nc.dram_tensor("wk", [P, KC, FPG], F32R, kind="ExternalInput")
    wv = nc.dram_tensor("wv", [P, KC, FPG], F16, kind="ExternalInput")
    woT = nc.dram_tensor("woT", [P, KC, FPG], F16, kind="ExternalInput")
    bq = nc.dram_tensor("bq", [P, HPG], F32, kind="ExternalInput")
    bk = nc.dram_tensor("bk", [P, HPG], F32, kind="ExternalInput")
    bvb = nc.dram_tensor("bvb", [P, FPG], F32, kind="ExternalInput")
    bob = nc.dram_tensor("bob", [P, FPG], F32, kind="ExternalInput")
    maskh = nc.dram_tensor("maskh", [P, P], F16, kind="ExternalInput")
    identd = nc.dram_tensor("identd", [P, P], F16, kind="ExternalInput")
    out = nc.dram_tensor("out", [L, FPG], F32, kind="ExternalOutput")
    if variant == "timing":
        chain = nc.dram_tensor("chain", [1, 8], F32, kind="ExternalInput")
        dummy = nc.dram_tensor("chaino", [1, 8], F32, kind="ExternalOutput")

    with tile.TileContext(nc) as tc:
        with ExitStack() as ctx:
            consts = ctx.enter_context(tc.tile_pool(name="consts", bufs=1))

            maskh_t = consts.tile([P, P], F16)
            id_t = consts.tile([P, P], F16)
            bq_t = consts.tile([P, HPG], F32)
            bk_t = consts.tile([P, HPG], F32)
            bvb_t = consts.tile([P, FPG], F32)
            nc.scalar.dma_start(bvb_t[:], bvb[:])
            nc.scalar.dma_start(bq_t[:], bq[:])
            nc.scalar.dma_start(bk_t[:], bk[:])
            bob_t = consts.tile([P, FPG], F32)
            if variant == "timing":
                ch_t = consts.tile([1, 8], F32)
                nc.sync.dma_start(ch_t[:], chain[:])
                nc.sync.dma_start(dummy[:], ch_t[:])

            NREP = {"x4": 4, "x2": 2, "x2nc": 2}.get(variant, 1)
            for _rep in range(NREP):
                ag_outs = []
                with tc.tile_pool(name="qkv", bufs=1) as qkv:
                    kth = qkv.tile([P, HPG, L], F32R)   # (dh, head, j)
                    vt = qkv.tile([P, IB, FPG], F16)    # (j%128, blk, feat)

                    with tc.tile_pool(name="wqk", bufs=1) as wqk, \
                         tc.tile_pool(name="xqk", bufs=8) as xqkp, \
                         tc.tile_pool(name="qpan", bufs=2) as qpanp, \
                         tc.tile_pool(name="ppsum", bufs=1,
                                      space="PSUM") as ppsum:
                        wq_t = wqk.tile([P, KC, FPG], F32R)
                        wk_t = wqk.tile([P, KC, FPG], F32R)

                        # ---- phase 1: V projection (fp16) ----
                        with tc.tile_pool(name="wvp", bufs=1) as wvp, \
                             tc.tile_pool(name="xvp", bufs=2) as xvp, \
                             tc.tile_pool(name="vpsum", bufs=2,
                                          space="PSUM") as vpsum:
                            wv_t = wvp.tile([P, KC, FPG], F16)
                            for ip in range(NPAN):
                                xv_t = xvp.tile([P, KC, PAN], F16, tag="xv")
                                if ip == 0:
                                    # interleave w/x quarters so the first
                                    # matmul's operands land first
                                    for xc in range(4):
                                        ksl = slice(xc * 4, xc * 4 + 4)
                                        nc.sync.dma_start(xv_t[:, ksl],
                                                          xv[:, ip, ksl])
                                        nc.sync.dma_start(
                                            wv_t[:, ksl], wv[:, ksl])
                                else:
                                    nc.sync.dma_start(xv_t[:], xv[:, ip])
                                if 1 <= ip <= 4:
                                    # prefetch K/Q weights on the scalar
                                    # queue, quartered so xv panels interleave
                                    # at the (serial) DMA transfer resource
                                    ksl = slice((ip - 1) * 4, ip * 4)
                                    nc.scalar.dma_start(wk_t[:, ksl],
                                                        wk[:, ksl])
                                if 3 <= ip <= 6:
                                    ksl = slice((ip - 3) * 4, (ip - 2) * 4)
                                    nc.scalar.dma_start(wq_t[:, ksl],
                                                        wq[:, ksl])
                                if ip == 5 and _rep == 0:
                                    nc.scalar.dma_start(maskh_t[:], maskh[:])
                                    nc.scalar.dma_start(id_t[:], identd[:])
                                    nc.scalar.dma_start(bob_t[:], bob[:])
                                for sub in range(PAN // P):
                                    ib = ip * (PAN // P) + sub
                                    ps = vpsum.tile([P, FPG], F32, tag="pv")
                                    for kc in range(KC):
                                        nc.tensor.matmul(
                                            ps[:],
                                            xv_t[:, kc, sub * P:(sub + 1) * P],
                                            wv_t[:, kc, :],
                                            start=(kc == 0),
                                            stop=(kc == KC - 1))
                                    nc.vector.tensor_tensor(
                                        vt[:, ib, :], ps[:], bvb_t[:],
                                        op=OP.add)

                        # ---- K/Q projection panel subtasks ----
                        xtiles = {}
                        qpan_tiles = {}

                        def issue_x(kind, p):
                            """Issue the x-panel load for (kind, p) as 4
                            quarter tiles (0.53MB each). Quarter-granular
                            pool rotation keeps the WAR semaphore of each
                            DMA ~2 panels back, so a blocked transfer never
                            head-of-line-blocks the Pool queue for long."""
                            src = xq if kind == "q" else xk
                            qts = []
                            for qc in range(4):
                                ksl = slice(qc * 4, (qc + 1) * 4)
                                qt = xqkp.tile([P, 4, PAN], F32R, tag="xh",
                                               name=f"x{kind}{p}_{qc}")
                                nc.gpsimd.dma_start(qt[:], src[:, p, ksl])
                                qts.append(qt)
                            xtiles[(kind, p)] = qts

                        def emit_panel(kind, p, h):
                            qts = xtiles[(kind, p)]
                            if kind == "q" and p not in qpan_tiles:
                                qpan_tiles[p] = qpanp.tile(
                                    [P, HPG, PAN], F32R, tag="qp",
                                    name=f"qp{p}")
                            w_t = wq_t if kind == "q" else wk_t
                            bias_t = bq_t if kind == "q" else bk_t
                            fsl = slice(h * P, (h + 1) * P)
                            ps = ppsum.tile([P, 512], F32, tag="pp")
                            for kc in range(KC):
                                nc.tensor.matmul(
                                    ps[:, :PAN], w_t[:, kc, fsl],
                                    qts[kc // 4][:, kc % 4, :],
                                    start=(kc == 0), stop=(kc == KC - 1))
                            if kind == "q":
                                dst = qpan_tiles[p][:, h, :]
                            else:
                                dst = kth[:, h, p * PAN:(p + 1) * PAN]
                            nc.scalar.activation(
                                dst, ps[:, :PAN], ACTF.Identity,
                                bias=bias_t[:, h:h + 1], scale=1.0)

                        # prologue: panel 0 of K and Q; x for panels 0-1
                        issue_x("k", 0)
                        issue_x("q", 0)
                        for h in range(HPG):
                            emit_panel("k", 0, h)
                        issue_x("k", 1)
                        for h in range(HPG):
                            emit_panel("q", 0, h)
                        issue_x("q", 1)
                        # panel-2 x rides during group 0 (slots 3 and 7)

                        # ---- phase 2: attention, panel-major ----
                        with tc.tile_pool(name="otpool", bufs=1) as otpool, \
                             tc.tile_pool(name="spsum", bufs=5,
                                          space="PSUM") as spsum, \
                             tc.tile_pool(name="opsum", bufs=2,
                                          space="PSUM") as opsum, \
                             tc.tile_pool(name="ppool", bufs=4) as ppool, \
                             tc.tile_pool(name="ptpool", bufs=4) as ptpool, \
                             tc.tile_pool(name="stats", bufs=8) as stats, \
                             tc.tile_pool(name="dramio", bufs=1,
                                          space="DRAM") as dramio:

                            ots = [otpool.tile([P, L], F16, name=f"ot{hh}")
                                   for hh in range(HPG)]

                            def emit_S(h, ib):
                                nj = (ib + 1) * P
                                widths = _chunk_widths(nj)
                                isl = slice(ib * P, (ib + 1) * P)
                                mpart = stats.tile([P, 4], F32, tag="mp",
                                                   name=f"mp{h}_{ib}")
                                lpart = stats.tile([P, 4], F32, tag="lp",
                                                   name=f"lp{h}_{ib}")
                                qt = qpan_tiles[ib // 2]
                                qsl = qt[:, h,
                                         (ib % 2) * P:(ib % 2 + 1) * P]
                                chunks = []
                                off = 0
                                for jc, w in enumerate(widths):
                                    diag = jc == len(widths) - 1
                                    ps = spsum.tile([P, 512], F32, tag="s",
                                                    name=f"sps{h}_{ib}_{jc}")
                                    nc.tensor.matmul(
                                        ps[:, :w], qsl,
                                        kth[:, h, off:off + w],
                                        start=True, stop=not diag)
                                    if diag:
                                        # causal mask on the diagonal block,
                                        # accumulated on the PE: += I.T @ mask
                                        nc.tensor.matmul(
                                            ps[:, w - P:w], id_t[:],
                                            maskh_t[:],
                                            start=False, stop=True)
                                    chunks.append((ps, w, off, jc))
                                    off += w
                                return mpart, lpart, chunks

                            def emit_S_stats(e):
                                # scores arrive pre-scaled (host folds
                                # sqrt(scale) into Wq/Wk): negated chunk
                                # max IS the exp bias
                                mpart, lpart = e["mpart"], e["lpart"]
                                p_sb = ppool.tile([P, L], F16, tag="p",
                                                  name=f"p{e['h']}_{e['ib']}")
                                e["p_sb"] = p_sb
                                for ps, w, off, jc in e["chunks"]:
                                    nc.vector.reduce_max(
                                        mpart[:, jc:jc + 1], ps[:, :w],
                                        axis=AX.X, negate=True)
                                    nc.scalar.activation(
                                        p_sb[:, off:off + w], ps[:, :w],
                                        ACTF.Exp, bias=mpart[:, jc:jc + 1],
                                        scale=1.0,
                                        accum_out=lpart[:, jc:jc + 1])

                            def emit_scale_tr(e):
                                h, ib, p_sb, mpart, lpart, chunks = (
                                    e["h"], e["ib"], e["p_sb"], e["mpart"],
                                    e["lpart"], e["chunks"])
                                nch = len(chunks)
                                rmin = stats.tile([P, 1], F32, tag="nm",
                                                  name=f"nm{h}_{ib}")
                                nc.vector.tensor_reduce(
                                    rmin[:], mpart[:, :nch], axis=AX.X,
                                    op=OP.min)
                                # per-chunk correction c = exp(m_jc - m)
                                cfac = stats.tile([P, 4], F32, tag="cf",
                                                  name=f"cf{h}_{ib}")
                                nc.scalar.activation(
                                    cfac[:, :nch], mpart[:, :nch],
                                    ACTF.Exp, bias=rmin[:], scale=-1.0)
                                lw = stats.tile([P, 4], F32, tag="lw",
                                                name=f"lw{h}_{ib}")
                                nc.vector.tensor_tensor(
                                    lw[:, :nch], cfac[:, :nch],
                                    lpart[:, :nch], op=OP.mult)
                                lsum = stats.tile([P, 1], F32, tag="ls",
                                                  name=f"ls{h}_{ib}")
                                nc.vector.reduce_sum(lsum[:], lw[:, :nch],
                                                     axis=AX.X)
                                rinv = stats.tile([P, 1], F32, tag="ri",
                                                  name=f"ri{h}_{ib}")
                                nc.vector.reciprocal(rinv[:], lsum[:])
                                # P_jc *= c_jc * rinv, chunks alternating
                                # Pool/DVE so the stage's serial latency
                                # halves (Pool is SBUF-only; DVE runs 4x on
                                # packed fp16)
                                for ps, w, off, jc in chunks:
                                    # late groups: x-load DMAs are done, the
                                    # Pool queue is empty -> all pscales to
                                    # Pool, freeing DVE for the maxes
                                    if ib >= 10:
                                        eng = nc.gpsimd
                                    else:
                                        eng = nc.gpsimd if jc % 2 == 0 \
                                            else nc.vector
                                    eng.tensor_scalar(
                                        p_sb[:, off:off + w],
                                        p_sb[:, off:off + w],
                                        cfac[:, jc:jc + 1], rinv[:],
                                        op0=OP.mult, op1=OP.mult)
                                pt_sb = ptpool.tile([P, IB, P], F16,
                                                    tag="ptsb",
                                                    name=f"ptsb{h}_{ib}")
                                nblk = ib + 1
                                nc.sync.dma_start_transpose(
                                    pt_sb[:, :nblk, :], p_sb[:, :nblk * P])
                                e["pt_sb"] = pt_sb

                            def emit_av(e):
                                h, ib, pt_sb = e["h"], e["ib"], e["pt_sb"]
                                isl = slice(ib * P, (ib + 1) * P)
                                o_ps = opsum.tile([P, P], F32, tag="o",
                                                  name=f"o{h}_{ib}")
                                for jb in range(ib + 1):
                                    nc.tensor.matmul(
                                        o_ps[:],
                                        vt[:, jb, h * P:(h + 1) * P],
                                        pt_sb[:, jb, :],
                                        start=(jb == 0), stop=(jb == ib))
                                nc.vector.tensor_copy(
                                    ots[h][:, isl], o_ps[:])

                            # 4-stage pipeline: S(n) | stats(n-1) |
                            # scale+transpose(n-2) | AV(n-4) — each block's
                            # transpose gets ~2 block-iterations of wall
                            # time before its AV matmuls hit the PE
                            pend = []

                            def push_block(h, ib):
                                # stats/scale of older blocks FIRST: the
                                # spsum/ppool rotations must see those
                                # reader instructions before S(n) reuses
                                # their buffers (pool WAR is emission-order)
                                if len(pend) >= 1:
                                    emit_S_stats(pend[-1])
                                if len(pend) >= 2:
                                    emit_scale_tr(pend[-2])
                                mpart, lpart, chunks = emit_S(h, ib)
                                pend.append(dict(
                                    h=h, ib=ib, mpart=mpart,
                                    lpart=lpart, chunks=chunks))
                                while len(pend) > 4:
                                    emit_av(pend.pop(0))

                            for g in range(NGRP):
                                # late half of panel g's tasks (heads 2-3:
                                # their group-g blocks sit at slots 4-7) +
                                # early half of panel g+1's — spreads proj
                                # work one group later so group 7 isn't
                                # bare of PE filler
                                p = g + 1
                                tasks = []
                                if p < NPAN:
                                    tasks = ([("k", p, hh)
                                              for hh in range(HPG)] +
                                             [("q", p, hh)
                                              for hh in range(HPG)])
                                ti = 0
                                for h in range(HPG):
                                    for sub in range(2):
                                        ib = 2 * g + sub
                                        push_block(h, ib)
                                        if ti < len(tasks):
                                            emit_panel(*tasks[ti])
                                            ti += 1
                                        if h == 1 and sub == 1 \
                                                and g + 2 < NPAN:
                                            issue_x("k", g + 2)
                                        if h == 3 and sub == 1 \
                                                and g + 2 < NPAN:
                                            issue_x("q", g + 2)
                            emit_S_stats(pend[-1])
                            emit_scale_tr(pend[-2])
                            emit_scale_tr(pend[-1])
                            while pend:
                                emit_av(pend.pop(0))

                            # ---- gathers (all heads complete here) ----
                            if variant in ("nocoll", "x2nc"):
                                for h in range(HPG):
                                    ag_out = dramio.tile(
                                        [G, P, L], F16, tag=f"agout{h}",
                                        name=f"agout{h}")
                                    ag_outs.append(ag_out)
                                # seq-quarter-major so the Wo stream's first
                                # slices are ready after 2MB, not 8MB
                                for lq in range(4):
                                    lsl = slice(lq * 512, (lq + 1) * 512)
                                    for h in range(HPG):
                                        for gg in range(G):
                                            nc.sync.dma_start(
                                                ag_outs[h][gg][:, lsl],
                                                ots[h][:, lsl])
                            else:
                                for h in range(HPG):
                                    ag_in = dramio.tile(
                                        [P, L], F16, tag=f"agin{h}",
                                        name=f"agin{h}")
                                    nc.scalar.dma_start(ag_in[:], ots[h][:])
                                    ag_out = dramio.tile(
                                        [G, P, L], F16, tag=f"agout{h}",
                                        name=f"agout{h}")
                                    nc.gpsimd.collective_compute(
                                        "AllGather", OP.bypass,
                                        replica_groups=[[0, 1, 2, 3],
                                                        [4, 5, 6, 7]],
                                        ins=[ag_in[:].opt()],
                                        outs=[ag_out[:].opt()])
                                    ag_outs.append(ag_out)

                    # ---- phase 3: output projection, streamed in 256-seq
                    # slices (512B descriptors -> full DMA rate) ----
                    with tc.tile_pool(name="ph3", bufs=1) as ph3, \
                         tc.tile_pool(name="fapool", bufs=4) as fapool, \
                         tc.tile_pool(name="fopool", bufs=4) as fopool, \
                         tc.tile_pool(name="fpsum", bufs=8,
                                      space="PSUM") as fpsum:
                        wo_t = ph3.tile([P, KC, FPG], F16, name=f"wo{_rep}")
                        nc.scalar.dma_start(wo_t[:, :KC // 2],
                                            woT[:, :KC // 2])
                        nc.scalar.dma_start(wo_t[:, KC // 2:],
                                            woT[:, KC // 2:])
                        NQ = 8
                        QW = L // NQ   # 256 seq cols per slice
                        for q in range(NQ):
                            qsl = slice(q * QW, (q + 1) * QW)
                            atq = fapool.tile([P, HPG, G, QW], F16,
                                              tag="atq", name=f"atq{q}")
                            for hc in range(HPG):
                                nc.scalar.dma_start(
                                    atq[:, hc],
                                    ag_outs[hc].rearrange(
                                        "g p l -> p g l")[:, :, qsl])
                            pss = [fpsum.tile([P, FPG], F32, tag="f",
                                              name=f"fps{q}_{i}")
                                   for i in range(2)]
                            for hc in range(HPG):
                                for g_idx in range(G):
                                    for i in range(2):
                                        nc.tensor.matmul(
                                            pss[i][:],
                                            atq[:, hc, g_idx,
                                                i * P:(i + 1) * P],
                                            wo_t[:, g_idx * HPG + hc, :],
                                            start=(hc == 0 and g_idx == 0),
                                            stop=(hc == HPG - 1
                                                  and g_idx == G - 1))
                            for i in range(2):
                                ib = 2 * q + i
                                o_sb = fopool.tile([P, FPG], F32, tag="fo")
                                nc.vector.tensor_tensor(
                                    o_sb[:], pss[i][:], bob_t[:], op=OP.add)
                                nc.gpsimd.dma_start(
                                    out[ib * P:(ib + 1) * P, :], o_sb[:])

    nc.compile()
    return nc


def _tilex(x, dtype):
    # [D, L] -> [P, NPAN, KC, PAN]: (kc*128+p, pl*PAN+c) -> [p, pl, kc, c]
    return np.ascontiguousarray(
        x.reshape(KC, P, NPAN, PAN).transpose(1, 2, 0, 3).astype(dtype))


def _tilew(w, dtype):
    # [D, FPG] -> [P, KC, FPG]
    return np.ascontiguousarray(
        w.reshape(KC, P, FPG).transpose(1, 0, 2).astype(dtype))


def _prepare_in_maps(q, k, v, Wq, bq, Wk, bk, Wv, bv, Wo, bo):
    mask16 = np.where(
        np.arange(P)[None, :] > np.arange(P)[:, None],
        np.float16(-30000.0), np.float16(0.0)).astype(np.float16)
    ident = np.eye(P, dtype=np.float16)

    xs = {}
    for b in range(B):
        for nm, arr in (("q", q), ("k", k)):
            xs[(nm, b)] = _tilex(
                np.ascontiguousarray(arr[b].T, dtype=np.float32), np.float32)
        xs[("v", b)] = _tilex(
            np.ascontiguousarray(v[b].T, dtype=np.float32), np.float16)

    rs = np.float32(SCALE ** 0.5)
    in_maps = []
    for c in range(8):
        b, g = divmod(c, G)
        F = slice(g * FPG, (g + 1) * FPG)
        in_maps.append({
            "xq": xs[("q", b)],
            "xk": xs[("k", b)],
            "xv": xs[("v", b)],
            "wq": _tilew(
                np.ascontiguousarray(Wq[F, :].T, dtype=np.float32) * rs,
                np.float32),
            "wk": _tilew(
                np.ascontiguousarray(Wk[F, :].T, dtype=np.float32) * rs,
                np.float32),
            "wv": _tilew(
                np.ascontiguousarray(Wv[F, :].T, dtype=np.float32),
                np.float16),
            "woT": _tilew(
                np.ascontiguousarray(Wo[F, :].T, dtype=np.float32),
                np.float16),
            "bq": np.ascontiguousarray(
                (bq[F] * rs).astype(np.float32).reshape(HPG, P).T),
            "bk": np.ascontiguousarray(
                (bk[F] * rs).astype(np.float32).reshape(HPG, P).T),
            "bvb": np.broadcast_to(bv[F][None, :], (P, FPG)).astype(
                np.float32),
            "bob": np.broadcast_to(bo[F][None, :], (P, FPG)).astype(
                np.float32),
            "maskh": mask16,
            "identd": ident,
        })
    return in_maps


def kernel(**inputs) -> np.ndarray:
    global _COMPILED
    from concourse.bass_utils import run_bass_kernel_spmd

    if _COMPILED is None:
        _COMPILED = _build()
    nc = _COMPILED

    in_maps = _prepare_in_maps(**inputs)
    res = run_bass_kernel_spmd(nc, in_maps, list(range(8)))

    outp = np.empty((B, L, D), dtype=np.float32)
    for c in range(8):
        b, g = divmod(c, G)
        outp[b, :, g * FPG:(g + 1) * FPG] = res.results[c]["out"]
    return outp


if __name__ == "__main__":
    rng = np.random.default_rng(1)
    ins = {
        "q": rng.standard_normal((B, L, D), dtype=np.float32),
        "k": rng.standard_normal((B, L, D), dtype=np.float32),
        "v": rng.standard_normal((B, L, D), dtype=np.float32),
        "Wq": rng.standard_normal((D, D), dtype=np.float32) * 0.02,
        "bq": rng.standard_normal(D).astype(np.float32) * 0.02,
        "Wk": rng.standard_normal((D, D), dtype=np.float32) * 0.02,
        "bk": rng.standard_normal(D).astype(np.float32) * 0.02,
        "Wv": rng.standard_normal((D, D), dtype=np.float32) * 0.02,
        "bv": rng.standard_normal(D).astype(np.float32) * 0.02,
        "Wo": rng.standard_normal((D, D), dtype=np.float32) * 0.02,
        "bo": rng.standard_normal(D).astype(np.float32) * 0.02,
    }
    o = kernel(**ins)
    print("kernel ran, out shape", o.shape)


# revision 43
# speedup vs baseline: 1.0598x; 1.0084x over previous
"""Trainium2 Bass kernel for causal multi-head attention (B=2, L=2048, D=2048,
H=16 heads, DH=128), sharded over 8 NeuronCores.

Sharding: core c handles batch b=c//4 and head-group g=c%4 (4 heads = 512
features). The only cross-core communication is a per-head fp16 AllGather of
attention outputs within each 4-core batch group.

v3 design (fp32r + panel-major interleave):
- Q/K projections and the score matmuls run in float32r: the PE processes
  f32r at 1 col/cycle when the moving dim is >= 256 (same as fp16), while
  storage rounds to ~14 mantissa bits — accurate enough to drop the whole
  compensated fp16+fp8 DoubleRow correction scheme of v2 (S abs err ~0.02 vs
  the 0.109 budget).
- Attention runs PANEL-MAJOR: groups g=0..7 process seq blocks {2g, 2g+1} of
  all 4 heads. K/Q projection panel subtasks (one head x one 256-seq panel,
  16 matmuls) interleave one-per-block into group g for panel g+1, so the PE
  always has dense matmul work while the softmax chain (DVE max, ACT exp,
  Pool pscale, DMA transpose) drains. qth lives as 2 rotating 256-seq panel
  tiles; kth accumulates in a full 4MB f32r tile.
- Engine split: DVE = chunk maxes + stats; ACT = exp (accum_out row sums) +
  proj/o evacuations; Pool = pscales (Pool cannot touch PSUM); PE adds the
  causal mask via the id^T @ mask accumulation trick.
- DMA queues: sync = V-phase loads + P^T transposes + nocoll gather copies;
  scalar = consts + wq/wk prefetch + ag bounce + wo + atq; vector = xq/xk
  half-tile streams + out writes. Keeps the latency-critical transposes from
  queuing behind bulk loads.
- Wo phase reads gathered activations in 256-seq slices so every descriptor
  is 512B (full DMA rate; 128-seq slices pay the sub-512B 2x penalty).
- PSUM: 2 proj banks + 4 score banks + 2 O^T banks (each O^T accumulator
  must own a bank: concurrent groups sharing a bank corrupt via bank-granular
  has_written clears). Score chunks use widths >=256 (e.g. 384+256 instead
  of 512+128) to stay at the f32r fast rate.
"""
import sys

sys.path.insert(0, "/opt/trn_rl_repo")

import numpy as np

B, L, D, H = 2, 2048, 2048, 16
DH = D // H          # 128
G = 4                # head-groups (tensor-parallel degree per batch)
HPG = H // G         # heads per group = 4
FPG = HPG * DH       # features per group = 512
P = 128
SCALE = float(DH) ** 0.5
KC = D // P          # 16 contraction chunks
IB = L // P          # 16 seq blocks of 128
PAN = 256            # K/Q/V projection seq panel
NPAN = L // PAN      # 8
NGRP = IB // 2       # 8 block groups (2 blocks x 4 heads each)

_COMPILED = None


def _chunk_widths(nj):
    """Split nj into chunks of <=512 with every chunk >=256 when possible
    (f32r matmuls run 4x slower below a 256-wide moving dim)."""
    if nj <= 512:
        return [nj]
    nch = (nj + 511) // 512
    rem = nj - 512 * (nch - 1)
    w = [512] * (nch - 1) + [rem]
    if rem < 256:
        w[-2:] = [512 + rem - 256, 256]
    return w


def _build(variant="main"):
    import concourse.bacc as bacc
    import concourse.tile as tile
    from concourse import mybir
    from contextlib import ExitStack

    F32 = mybir.dt.float32
    F32R = mybir.dt.float32r
    F16 = mybir.dt.float16
    AX = mybir.AxisListType
    OP = mybir.AluOpType
    ACTF = mybir.ActivationFunctionType

    nc = bacc.Bacc("TRN2", target_bir_lowering=False, debug=False, num_devices=8)

    # ---- DRAM I/O (x/w pre-arranged host-side in tile order) ----
    xq = nc.dram_tensor("xq", [P, NPAN, KC, PAN], F32R, kind="ExternalInput")
    xk = nc.dram_tensor("xk", [P, NPAN, KC, PAN], F32R, kind="ExternalInput")
    xv = nc.dram_tensor("xv", [P, NPAN, KC, PAN], F16, kind="ExternalInput")
    wq = nc.dram_tensor("wq", [P, KC, FPG], F32R, kind="ExternalInput")
    wk = nc.dram_tensor("wk", [P, KC, FPG], F32R, kind="ExternalInput")
    wv = nc.dram_tensor("wv", [P, KC, FPG], F16, kind="ExternalInput")
    woT = nc.dram_tensor("woT", [P, KC, FPG], F16, kind="ExternalInput")
    bq = nc.dram_tensor("bq", [P, HPG], F32, kind="ExternalInput")
    bk = nc.dram_tensor("bk", [P, HPG], F32, kind="ExternalInput")
    bvb = nc.dram_tensor("bvb", [P, FPG], F32, kind="ExternalInput")
    bob = nc.dram_tensor("bob", [P, FPG], F32, kind="ExternalInput")
    maskh = nc.dram_tensor("maskh", [P, P], F16, kind="ExternalInput")
    identd = nc.dram_tensor("identd", [P, P], F16, kind="ExternalInput")
    out = nc.dram_tensor("out", [L, FPG], F32, kind="ExternalOutput")
    if variant == "timing":
        chain = nc.dram_tensor("chain", [1, 8], F32, kind="ExternalInput")
        dummy = nc.dram_tensor("chaino", [1, 8], F32, kind="ExternalOutput")

    with tile.TileContext(nc) as tc:
        with ExitStack() as ctx:
            consts = ctx.enter_context(tc.tile_pool(name="consts", bufs=1))

            maskh_t = consts.tile([P, P], F16)
            id_t = consts.tile([P, P], F16)
            bq_t = consts.tile([P, HPG], F32)
            bk_t = consts.tile([P, HPG], F32)
            bvb_t = consts.tile([P, FPG], F32)
            nc.scalar.dma_start(bvb_t[:], bvb[:])
            nc.scalar.dma_start(bq_t[:], bq[:])
            nc.scalar.dma_start(bk_t[:], bk[:])
            bob_t = consts.tile([P, FPG], F32)
            if variant == "timing":
                ch_t = consts.tile([1, 8], F32)
                nc.sync.dma_start(ch_t[:], chain[:])
                nc.sync.dma_start(dummy[:], ch_t[:])

            NREP = {"x4": 4, "x2": 2, "x2nc": 2}.get(variant, 1)
            for _rep in range(NREP):
                ag_outs = []
                with tc.tile_pool(name="qkv", bufs=1) as qkv:
                    kth = qkv.tile([P, HPG, L], F32R)   # (dh, head, j)
                    vt = qkv.tile([P, IB, FPG], F16)    # (j%128, blk, feat)

                    with tc.tile_pool(name="wqk", bufs=1) as wqk, \
                         tc.tile_pool(name="xqk", bufs=8) as xqkp, \
                         tc.tile_pool(name="qpan", bufs=2) as qpanp, \
                         tc.tile_pool(name="ppsum", bufs=1,
                                      space="PSUM") as ppsum:
                        wq_t = wqk.tile([P, KC, FPG], F32R)
                        wk_t = wqk.tile([P, KC, FPG], F32R)

                        # ---- phase 1: V projection (fp16) ----
                        with tc.tile_pool(name="wvp", bufs=1) as wvp, \
                             tc.tile_pool(name="xvp", bufs=2) as xvp, \
                             tc.tile_pool(name="vpsum", bufs=2,
                                          space="PSUM") as vpsum:
                            wv_t = wvp.tile([P, KC, FPG], F16)
                            for ip in range(NPAN):
                                xv_t = xvp.tile([P, KC, PAN], F16, tag="xv")
                                if ip == 0:
                                    # interleave w/x quarters so the first
                                    # matmul's operands land first
                                    for xc in range(4):
                                        ksl = slice(xc * 4, xc * 4 + 4)
                                        nc.sync.dma_start(xv_t[:, ksl],
                                                          xv[:, ip, ksl])
                                        nc.sync.dma_start(
                                            wv_t[:, ksl], wv[:, ksl])
                                else:
                                    nc.sync.dma_start(xv_t[:], xv[:, ip])
                                if 1 <= ip <= 4:
                                    # prefetch K/Q weights on the scalar
                                    # queue, quartered so xv panels interleave
                                    # at the (serial) DMA transfer resource
                                    ksl = slice((ip - 1) * 4, ip * 4)
                                    nc.scalar.dma_start(wk_t[:, ksl],
                                                        wk[:, ksl])
                                if 3 <= ip <= 6:
                                    ksl = slice((ip - 3) * 4, (ip - 2) * 4)
                                    nc.scalar.dma_start(wq_t[:, ksl],
                                                        wq[:, ksl])
                                if ip == 5 and _rep == 0:
                                    nc.scalar.dma_start(maskh_t[:], maskh[:])
                                    nc.scalar.dma_start(id_t[:], identd[:])
                                    nc.scalar.dma_start(bob_t[:], bob[:])
                                for sub in range(PAN // P):
                                    ib = ip * (PAN // P) + sub
                                    ps = vpsum.tile([P, FPG], F32, tag="pv")
                                    for kc in range(KC):
                                        nc.tensor.matmul(
                                            ps[:],
                                            xv_t[:, kc, sub * P:(sub + 1) * P],
                                            wv_t[:, kc, :],
                                            start=(kc == 0),
                                            stop=(kc == KC - 1))
                                    nc.vector.tensor_tensor(
                                        vt[:, ib, :], ps[:], bvb_t[:],
                                        op=OP.add)

                        # ---- K/Q projection panel subtasks ----
                        xtiles = {}
                        qpan_tiles = {}

                        def issue_x(kind, p):
                            """Issue the x-panel load for (kind, p) as 4
                            quarter tiles (0.53MB each). Quarter-granular
                            pool rotation keeps the WAR semaphore of each
                            DMA ~2 panels back, so a blocked transfer never
                            head-of-line-blocks the Pool queue for long."""
                            src = xq if kind == "q" else xk
                            qts = []
                            for qc in range(4):
                                ksl = slice(qc * 4, (qc + 1) * 4)
                                qt = xqkp.tile([P, 4, PAN], F32R, tag="xh",
                                               name=f"x{kind}{p}_{qc}")
                                nc.gpsimd.dma_start(qt[:], src[:, p, ksl])
                                qts.append(qt)
                            xtiles[(kind, p)] = qts

                        def emit_panel(kind, p, h):
                            qts = xtiles[(kind, p)]
                            if kind == "q" and p not in qpan_tiles:
                                qpan_tiles[p] = qpanp.tile(
                                    [P, HPG, PAN], F32R, tag="qp",
                                    name=f"qp{p}")
                            w_t = wq_t if kind == "q" else wk_t
                            bias_t = bq_t if kind == "q" else bk_t
                            fsl = slice(h * P, (h + 1) * P)
                            ps = ppsum.tile([P, 512], F32, tag="pp")
                            for kc in range(KC):
                                nc.tensor.matmul(
                                    ps[:, :PAN], w_t[:, kc, fsl],
                                    qts[kc // 4][:, kc % 4, :],
                                    start=(kc == 0), stop=(kc == KC - 1))
                            if kind == "q":
                                dst = qpan_tiles[p][:, h, :]
                            else:
                                dst = kth[:, h, p * PAN:(p + 1) * PAN]
                            nc.scalar.activation(
                                dst, ps[:, :PAN], ACTF.Identity,
                                bias=bias_t[:, h:h + 1], scale=1.0)

                        # prologue: panel 0 of K and Q; x for panels 0-1
                        issue_x("k", 0)
                        issue_x("q", 0)
                        for h in range(HPG):
                            emit_panel("k", 0, h)
                        issue_x("k", 1)
                        for h in range(HPG):
                            emit_panel("q", 0, h)
                        issue_x("q", 1)
                        # panel-2 x rides during group 0 (slots 3 and 7)

                        # ---- phase 2: attention, panel-major ----
                        with tc.tile_pool(name="otpool", bufs=1) as otpool, \
                             tc.tile_pool(name="spsum", bufs=5,
                                          space="PSUM") as spsum, \
                             tc.tile_pool(name="opsum", bufs=2,
                                          space="PSUM") as opsum, \
                             tc.tile_pool(name="ppool", bufs=4) as ppool, \
                             tc.tile_pool(name="ptpool", bufs=4) as ptpool, \
                             tc.tile_pool(name="stats", bufs=8) as stats, \
                             tc.tile_pool(name="dramio", bufs=1,
                                          space="DRAM") as dramio:

                            ots = [otpool.tile([P, L], F16, name=f"ot{hh}")
                                   for hh in range(HPG)]

                            def emit_S(h, ib):
                                nj = (ib + 1) * P
                                widths = _chunk_widths(nj)
                                isl = slice(ib * P, (ib + 1) * P)
                                mpart = stats.tile([P, 4], F32, tag="mp",
                                                   name=f"mp{h}_{ib}")
                                lpart = stats.tile([P, 4], F32, tag="lp",
                                                   name=f"lp{h}_{ib}")
                                qt = qpan_tiles[ib // 2]
                                qsl = qt[:, h,
                                         (ib % 2) * P:(ib % 2 + 1) * P]
                                chunks = []
                                off = 0
                                for jc, w in enumerate(widths):
                                    diag = jc == len(widths) - 1
                                    ps = spsum.tile([P, 512], F32, tag="s",
                                                    name=f"sps{h}_{ib}_{jc}")
                                    nc.tensor.matmul(
                                        ps[:, :w], qsl,
                                        kth[:, h, off:off + w],
                                        start=True, stop=not diag)
                                    if diag:
                                        # causal mask on the diagonal block,
                                        # accumulated on the PE: += I.T @ mask
                                        nc.tensor.matmul(
                                            ps[:, w - P:w], id_t[:],
                                            maskh_t[:],
                                            start=False, stop=True)
                                    chunks.append((ps, w, off, jc))
                                    off += w
                                return mpart, lpart, chunks

                            def emit_S_stats(e):
                                # scores arrive pre-scaled (host folds
                                # sqrt(scale) into Wq/Wk): negated chunk
                                # max IS the exp bias
                                mpart, lpart = e["mpart"], e["lpart"]
                                p_sb = ppool.tile([P, L], F16, tag="p",
                                                  name=f"p{e['h']}_{e['ib']}")
                                e["p_sb"] = p_sb
                                for ps, w, off, jc in e["chunks"]:
                                    nc.vector.reduce_max(
                                        mpart[:, jc:jc + 1], ps[:, :w],
                                        axis=AX.X, negate=True)
                                    nc.scalar.activation(
                                        p_sb[:, off:off + w], ps[:, :w],
                                        ACTF.Exp, bias=mpart[:, jc:jc + 1],
                                        scale=1.0,
                                        accum_out=lpart[:, jc:jc + 1])

                            def emit_scale_tr(e):
                                h, ib, p_sb, mpart, lpart, chunks = (
                                    e["h"], e["ib"], e["p_sb"], e["mpart"],
                                    e["lpart"], e["chunks"])
                                nch = len(chunks)
                                rmin = stats.tile([P, 1], F32, tag="nm",
                                                  name=f"nm{h}_{ib}")
                                nc.vector.tensor_reduce(
                                    rmin[:], mpart[:, :nch], axis=AX.X,
                                    op=OP.min)
                                # per-chunk correction c = exp(m_jc - m)
                                cfac = stats.tile([P, 4], F32, tag="cf",
                                                  name=f"cf{h}_{ib}")
                                nc.scalar.activation(
                                    cfac[:, :nch], mpart[:, :nch],
                                    ACTF.Exp, bias=rmin[:], scale=-1.0)
                                lw = stats.tile([P, 4], F32, tag="lw",
                                                name=f"lw{h}_{ib}")
                                nc.vector.tensor_tensor(
                                    lw[:, :nch], cfac[:, :nch],
                                    lpart[:, :nch], op=OP.mult)
                                lsum = stats.tile([P, 1], F32, tag="ls",
                                                  name=f"ls{h}_{ib}")
                                nc.vector.reduce_sum(lsum[:], lw[:, :nch],
                                                     axis=AX.X)
                                rinv = stats.tile([P, 1], F32, tag="ri",
                                                  name=f"ri{h}_{ib}")
                                nc.vector.reciprocal(rinv[:], lsum[:])
                                # P_jc *= c_jc * rinv, chunks alternating
                                # Pool/DVE so the stage's serial latency
                                # halves (Pool is SBUF-only; DVE runs 4x on
                                # packed fp16)
                                for ps, w, off, jc in chunks:
                                    # late groups: x-load DMAs are done, the
                                    # Pool queue is empty -> all pscales to
                                    # Pool, freeing DVE for the maxes
                                    if ib >= 10:
                                        eng = nc.gpsimd
                                    else:
                                        eng = nc.gpsimd if jc % 2 == 0 \
                                            else nc.vector
                                    eng.tensor_scalar(
                                        p_sb[:, off:off + w],
                                        p_sb[:, off:off + w],
                                        cfac[:, jc:jc + 1], rinv[:],
                                        op0=OP.mult, op1=OP.mult)
                                pt_sb = ptpool.tile([P, IB, P], F16,
                                                    tag="ptsb",
                                                    name=f"ptsb{h}_{ib}")
                                nblk = ib + 1
                                nc.sync.dma_start_transpose(
                                    pt_sb[:, :nblk, :], p_sb[:, :nblk * P])
                                e["pt_sb"] = pt_sb

                            def emit_av(e):
                                h, ib, pt_sb = e["h"], e["ib"], e["pt_sb"]
                                isl = slice(ib * P, (ib + 1) * P)
                                o_ps = opsum.tile([P, P], F32, tag="o",
                                                  name=f"o{h}_{ib}")
                                for jb in range(ib + 1):
                                    nc.tensor.matmul(
                                        o_ps[:],
                                        vt[:, jb, h * P:(h + 1) * P],
                                        pt_sb[:, jb, :],
                                        start=(jb == 0), stop=(jb == ib))
                                nc.vector.tensor_copy(
                                    ots[h][:, isl], o_ps[:])

                            # 4-stage pipeline: S(n) | stats(n-1) |
                            # scale+transpose(n-2) | AV(n-4) — each block's
                            # transpose gets ~2 block-iterations of wall
                            # time before its AV matmuls hit the PE
                            pend = []

                            def push_block(h, ib):
                                # stats/scale of older blocks FIRST: the
                                # spsum/ppool rotations must see those
                                # reader instructions before S(n) reuses
                                # their buffers (pool WAR is emission-order)
                                if len(pend) >= 1:
                                    emit_S_stats(pend[-1])
                                if len(pend) >= 2:
                                    emit_scale_tr(pend[-2])
                                mpart, lpart, chunks = emit_S(h, ib)
                                pend.append(dict(
                                    h=h, ib=ib, mpart=mpart,
                                    lpart=lpart, chunks=chunks))
                                while len(pend) > 5:
                                    emit_av(pend.pop(0))

                            for g in range(NGRP):
                                # late half of panel g's tasks (heads 2-3:
                                # their group-g blocks sit at slots 4-7) +
                                # early half of panel g+1's — spreads proj
                                # work one group later so group 7 isn't
                                # bare of PE filler
                                p = g + 1
                                tasks = []
                                if p < NPAN:
                                    tasks = ([("k", p, hh)
                                              for hh in range(HPG)] +
                                             [("q", p, hh)
                                              for hh in range(HPG)])
                                ti = 0
                                for h in range(HPG):
                                    for sub in range(2):
                                        ib = 2 * g + sub
                                        push_block(h, ib)
                                        if ti < len(tasks):
                                            emit_panel(*tasks[ti])
                                            ti += 1
                                        if h == 1 and sub == 1 \
                                                and g + 2 < NPAN:
                                            issue_x("k", g + 2)
                                        if h == 3 and sub == 1 \
                                                and g + 2 < NPAN:
                                            issue_x("q", g + 2)
                            emit_S_stats(pend[-1])
                            emit_scale_tr(pend[-2])
                            emit_scale_tr(pend[-1])
                            while pend:
                                emit_av(pend.pop(0))

                            # ---- gathers (all heads complete here) ----
                            if variant in ("nocoll", "x2nc"):
                                for h in range(HPG):
                                    ag_out = dramio.tile(
                                        [G, P, L], F16, tag=f"agout{h}",
                                        name=f"agout{h}")
                                    ag_outs.append(ag_out)
                                # seq-quarter-major so the Wo stream's first
                                # slices are ready after 2MB, not 8MB
                                for lq in range(4):
                                    lsl = slice(lq * 512, (lq + 1) * 512)
                                    for h in range(HPG):
                                        for gg in range(G):
                                            nc.sync.dma_start(
                                                ag_outs[h][gg][:, lsl],
                                                ots[h][:, lsl])
                            else:
                                for h in range(HPG):
                                    ag_in = dramio.tile(
                                        [P, L], F16, tag=f"agin{h}",
                                        name=f"agin{h}")
                                    nc.scalar.dma_start(ag_in[:], ots[h][:])
                                    ag_out = dramio.tile(
                                        [G, P, L], F16, tag=f"agout{h}",
                                        name=f"agout{h}")
                                    nc.gpsimd.collective_compute(
                                        "AllGather", OP.bypass,
                                        replica_groups=[[0, 1, 2, 3],
                                                        [4, 5, 6, 7]],
                                        ins=[ag_in[:].opt()],
                                        outs=[ag_out[:].opt()])
                                    ag_outs.append(ag_out)

                    # ---- phase 3: output projection, streamed in 256-seq
                    # slices (512B descriptors -> full DMA rate) ----
                    with tc.tile_pool(name="ph3", bufs=1) as ph3, \
                         tc.tile_pool(name="fapool", bufs=4) as fapool, \
                         tc.tile_pool(name="fopool", bufs=4) as fopool, \
                         tc.tile_pool(name="fpsum", bufs=8,
                                      space="PSUM") as fpsum:
                        wo_t = ph3.tile([P, KC, FPG], F16, name=f"wo{_rep}")
                        nc.scalar.dma_start(wo_t[:, :KC // 2],
                                            woT[:, :KC // 2])
                        nc.scalar.dma_start(wo_t[:, KC // 2:],
                                            woT[:, KC // 2:])
                        NQ = 8
                        QW = L // NQ   # 256 seq cols per slice
                        for q in range(NQ):
                            qsl = slice(q * QW, (q + 1) * QW)
                            atq = fapool.tile([P, HPG, G, QW], F16,
                                              tag="atq", name=f"atq{q}")
                            for hc in range(HPG):
                                nc.scalar.dma_start(
                                    atq[:, hc],
                                    ag_outs[hc].rearrange(
                                        "g p l -> p g l")[:, :, qsl])
                            pss = [fpsum.tile([P, FPG], F32, tag="f",
                                              name=f"fps{q}_{i}")
                                   for i in range(2)]
                            for hc in range(HPG):
                                for g_idx in range(G):
                                    for i in range(2):
                                        nc.tensor.matmul(
                                            pss[i][:],
                                            atq[:, hc, g_idx,
                                                i * P:(i + 1) * P],
                                            wo_t[:, g_idx * HPG + hc, :],
                                            start=(hc == 0 and g_idx == 0),
                                            stop=(hc == HPG - 1
                                                  and g_idx == G - 1))
                            for i in range(2):
                                ib = 2 * q + i
                                o_sb = fopool.tile([P, FPG], F32, tag="fo")
                                nc.vector.tensor_tensor(
                                    o_sb[:], pss[i][:], bob_t[:], op=OP.add)
                                nc.gpsimd.dma_start(
                                    out[ib * P:(ib + 1) * P, :], o_sb[:])

    nc.compile()
    return nc


def _tilex(x, dtype):
    # [D, L] -> [P, NPAN, KC, PAN]: (kc*128+p, pl*PAN+c) -> [p, pl, kc, c]
    return np.ascontiguousarray(
        x.reshape(KC, P, NPAN, PAN).transpose(1, 2, 0, 3).astype(dtype))


def _tilew(w, dtype):
    # [D, FPG] -> [P, KC, FPG]
    return np.ascontiguousarray(
        w.reshape(KC, P, FPG).transpose(1, 0, 2).astype(dtype))


def _prepare_in_maps(q, k, v, Wq, bq, Wk, bk, Wv, bv, Wo, bo):
    mask16 = np.where(
        np.arange(P)[None, :] > np.arange(P)[:, None],
        np.float16(-30000.0), np.float16(0.0)).astype(np.float16)
    ident = np.eye(P, dtype=np.float16)

    xs = {}
    for b in range(B):
        for nm, arr in (("q", q), ("k", k)):
            xs[(nm, b)] = _tilex(
                np.ascontiguousarray(arr[b].T, dtype=np.float32), np.float32)
        xs[("v", b)] = _tilex(
            np.ascontiguousarray(v[b].T, dtype=np.float32), np.float16)

    rs = np.float32(SCALE ** 0.5)
    in_maps = []
    for c in range(8):
        b, g = divmod(c, G)
        F = slice(g * FPG, (g + 1) * FPG)
        in_maps.append({
            "xq": xs[("q", b)],
            "xk": xs[("k", b)],
            "xv": xs[("v", b)],
            "wq": _tilew(
                np.ascontiguousarray(Wq[F, :].T, dtype=np.float32) * rs,
                np.float32),
            "wk": _tilew(
                np.ascontiguousarray(Wk[F, :].T, dtype=np.float32) * rs,
                np.float32),
            "wv": _tilew(
                np.ascontiguousarray(Wv[F, :].T, dtype=np.float32),
                np.float16),
            "woT": _tilew(
                np.ascontiguousarray(Wo[F, :].T, dtype=np.float32),
                np.float16),
            "bq": np.ascontiguousarray(
                (bq[F] * rs).astype(np.float32).reshape(HPG, P).T),
            "bk": np.ascontiguousarray(
                (bk[F] * rs).astype(np.float32).reshape(HPG, P).T),
            "bvb": np.broadcast_to(bv[F][None, :], (P, FPG)).astype(
                np.float32),
            "bob": np.broadcast_to(bo[F][None, :], (P, FPG)).astype(
                np.float32),
            "maskh": mask16,
            "identd": ident,
        })
    return in_maps


def kernel(**inputs) -> np.ndarray:
    global _COMPILED
    from concourse.bass_utils import run_bass_kernel_spmd

    if _COMPILED is None:
        _COMPILED = _build()
    nc = _COMPILED

    in_maps = _prepare_in_maps(**inputs)
    res = run_bass_kernel_spmd(nc, in_maps, list(range(8)))

    outp = np.empty((B, L, D), dtype=np.float32)
    for c in range(8):
        b, g = divmod(c, G)
        outp[b, :, g * FPG:(g + 1) * FPG] = res.results[c]["out"]
    return outp


if __name__ == "__main__":
    rng = np.random.default_rng(1)
    ins = {
        "q": rng.standard_normal((B, L, D), dtype=np.float32),
        "k": rng.standard_normal((B, L, D), dtype=np.float32),
        "v": rng.standard_normal((B, L, D), dtype=np.float32),
        "Wq": rng.standard_normal((D, D), dtype=np.float32) * 0.02,
        "bq": rng.standard_normal(D).astype(np.float32) * 0.02,
        "Wk": rng.standard_normal((D, D), dtype=np.float32) * 0.02,
        "bk": rng.standard_normal(D).astype(np.float32) * 0.02,
        "Wv": rng.standard_normal((D, D), dtype=np.float32) * 0.02,
        "bv": rng.standard_normal(D).astype(np.float32) * 0.02,
        "Wo": rng.standard_normal((D, D), dtype=np.float32) * 0.02,
        "bo": rng.standard_normal(D).astype(np.float32) * 0.02,
    }
    o = kernel(**ins)
    print("kernel ran, out shape", o.shape)


# revision 52
# speedup vs baseline: 1.0803x; 1.0193x over previous
"""Trainium2 Bass kernel for causal multi-head attention (B=2, L=2048, D=2048,
H=16 heads, DH=128), sharded over 8 NeuronCores.

Sharding: core c handles batch b=c//4 and head-group g=c%4 (4 heads = 512
features). The only cross-core communication is a per-head fp16 AllGather of
attention outputs within each 4-core batch group.

v3 design (fp32r + panel-major interleave):
- Q/K projections and the score matmuls run in float32r: the PE processes
  f32r at 1 col/cycle when the moving dim is >= 256 (same as fp16), while
  storage rounds to ~14 mantissa bits — accurate enough to drop the whole
  compensated fp16+fp8 DoubleRow correction scheme of v2 (S abs err ~0.02 vs
  the 0.109 budget).
- Attention runs PANEL-MAJOR: groups g=0..7 process seq blocks {2g, 2g+1} of
  all 4 heads. K/Q projection panel subtasks (one head x one 256-seq panel,
  16 matmuls) interleave one-per-block into group g for panel g+1, so the PE
  always has dense matmul work while the softmax chain (DVE max, ACT exp,
  Pool pscale, DMA transpose) drains. qth lives as 2 rotating 256-seq panel
  tiles; kth accumulates in a full 4MB f32r tile.
- Engine split: DVE = chunk maxes + stats; ACT = exp (accum_out row sums) +
  proj/o evacuations; Pool = pscales (Pool cannot touch PSUM); PE adds the
  causal mask via the id^T @ mask accumulation trick.
- DMA queues: sync = V-phase loads + P^T transposes + nocoll gather copies;
  scalar = consts + wq/wk prefetch + ag bounce + wo + atq; vector = xq/xk
  half-tile streams + out writes. Keeps the latency-critical transposes from
  queuing behind bulk loads.
- Wo phase reads gathered activations in 256-seq slices so every descriptor
  is 512B (full DMA rate; 128-seq slices pay the sub-512B 2x penalty).
- PSUM: 2 proj banks + 4 score banks + 2 O^T banks (each O^T accumulator
  must own a bank: concurrent groups sharing a bank corrupt via bank-granular
  has_written clears). Score chunks use widths >=256 (e.g. 384+256 instead
  of 512+128) to stay at the f32r fast rate.
"""
import sys

sys.path.insert(0, "/opt/trn_rl_repo")

import numpy as np

B, L, D, H = 2, 2048, 2048, 16
DH = D // H          # 128
G = 4                # head-groups (tensor-parallel degree per batch)
HPG = H // G         # heads per group = 4
FPG = HPG * DH       # features per group = 512
P = 128
SCALE = float(DH) ** 0.5
KC = D // P          # 16 contraction chunks
IB = L // P          # 16 seq blocks of 128
PAN = 256            # K/Q/V projection seq panel
NPAN = L // PAN      # 8
NGRP = IB // 2       # 8 block groups (2 blocks x 4 heads each)

_COMPILED = None


def _chunk_widths(nj):
    """Split nj into chunks of <=512 with every chunk >=256 when possible
    (f32r matmuls run 4x slower below a 256-wide moving dim)."""
    if nj <= 512:
        return [nj]
    nch = (nj + 511) // 512
    rem = nj - 512 * (nch - 1)
    w = [512] * (nch - 1) + [rem]
    if rem < 256:
        w[-2:] = [512 + rem - 256, 256]
    return w


def _build(variant="main"):
    import concourse.bacc as bacc
    import concourse.tile as tile
    from concourse import mybir
    from contextlib import ExitStack

    F32 = mybir.dt.float32
    F32R = mybir.dt.float32r
    F16 = mybir.dt.float16
    AX = mybir.AxisListType
    OP = mybir.AluOpType
    ACTF = mybir.ActivationFunctionType

    nc = bacc.Bacc("TRN2", target_bir_lowering=False, debug=False, num_devices=8)

    # ---- DRAM I/O (x/w pre-arranged host-side in tile order) ----
    xq = nc.dram_tensor("xq", [P, NPAN, KC, PAN], F32R, kind="ExternalInput")
    xk = nc.dram_tensor("xk", [P, NPAN, KC, PAN], F32R, kind="ExternalInput")
    xv = nc.dram_tensor("xv", [P, NPAN, KC, PAN], F16, kind="ExternalInput")
    wq = nc.dram_tensor("wq", [P, KC, FPG], F32R, kind="ExternalInput")
    wk = nc.dram_tensor("wk", [P, KC, FPG], F32R, kind="ExternalInput")
    wv = nc.dram_tensor("wv", [P, KC, FPG], F16, kind="ExternalInput")
    woT = nc.dram_tensor("woT", [P, KC, FPG], F16, kind="ExternalInput")
    bq = nc.dram_tensor("bq", [P, HPG], F32, kind="ExternalInput")
    bk = nc.dram_tensor("bk", [P, HPG], F32, kind="ExternalInput")
    bvb = nc.dram_tensor("bvb", [P, FPG], F32, kind="ExternalInput")
    bob = nc.dram_tensor("bob", [P, FPG], F32, kind="ExternalInput")
    maskh = nc.dram_tensor("maskh", [P, P], F16, kind="ExternalInput")
    identd = nc.dram_tensor("identd", [P, P], F16, kind="ExternalInput")
    out = nc.dram_tensor("out", [L, FPG], F32, kind="ExternalOutput")
    if variant == "timing":
        chain = nc.dram_tensor("chain", [1, 8], F32, kind="ExternalInput")
        dummy = nc.dram_tensor("chaino", [1, 8], F32, kind="ExternalOutput")

    with tile.TileContext(nc) as tc:
        with ExitStack() as ctx:
            consts = ctx.enter_context(tc.tile_pool(name="consts", bufs=1))

            maskh_t = consts.tile([P, P], F16)
            id_t = consts.tile([P, P], F16)
            bq_t = consts.tile([P, HPG], F32)
            bk_t = consts.tile([P, HPG], F32)
            bvb_t = consts.tile([P, FPG], F32)
            nc.scalar.dma_start(bvb_t[:], bvb[:])
            nc.scalar.dma_start(bq_t[:], bq[:])
            nc.scalar.dma_start(bk_t[:], bk[:])
            bob_t = consts.tile([P, FPG], F32)
            if variant == "timing":
                ch_t = consts.tile([1, 8], F32)
                nc.sync.dma_start(ch_t[:], chain[:])
                nc.sync.dma_start(dummy[:], ch_t[:])

            NREP = {"x4": 4, "x2": 2, "x2nc": 2}.get(variant, 1)
            for _rep in range(NREP):
                ag_outs = []
                with tc.tile_pool(name="qkv", bufs=1) as qkv:
                    kth = qkv.tile([P, HPG, L], F32R)   # (dh, head, j)
                    vt = qkv.tile([P, IB, FPG], F16)    # (j%128, blk, feat)

                    with tc.tile_pool(name="wqk", bufs=1) as wqk, \
                         tc.tile_pool(name="xqk", bufs=8) as xqkp, \
                         tc.tile_pool(name="qpan", bufs=2) as qpanp, \
                         tc.tile_pool(name="ppsum", bufs=1,
                                      space="PSUM") as ppsum:
                        wq_t = wqk.tile([P, KC, FPG], F32R)
                        wk_t = wqk.tile([P, KC, FPG], F32R)

                        # ---- phase 1: V projection (fp16) ----
                        with tc.tile_pool(name="wvp", bufs=1) as wvp, \
                             tc.tile_pool(name="xvp", bufs=2) as xvp, \
                             tc.tile_pool(name="vpsum", bufs=2,
                                          space="PSUM") as vpsum:
                            wv_t = wvp.tile([P, KC, FPG], F16)
                            for ip in range(NPAN):
                                xv_t = xvp.tile([P, KC, PAN], F16, tag="xv")
                                if ip == 0:
                                    # interleave w/x quarters so the first
                                    # matmul's operands land first
                                    for xc in range(4):
                                        ksl = slice(xc * 4, xc * 4 + 4)
                                        nc.sync.dma_start(xv_t[:, ksl],
                                                          xv[:, ip, ksl])
                                        nc.sync.dma_start(
                                            wv_t[:, ksl], wv[:, ksl])
                                else:
                                    nc.sync.dma_start(xv_t[:], xv[:, ip])
                                if 1 <= ip <= 4:
                                    # prefetch K/Q weights on the scalar
                                    # queue, quartered so xv panels interleave
                                    # at the (serial) DMA transfer resource
                                    ksl = slice((ip - 1) * 4, ip * 4)
                                    nc.scalar.dma_start(wk_t[:, ksl],
                                                        wk[:, ksl])
                                if 3 <= ip <= 6:
                                    ksl = slice((ip - 3) * 4, (ip - 2) * 4)
                                    nc.scalar.dma_start(wq_t[:, ksl],
                                                        wq[:, ksl])
                                if ip == 5 and _rep == 0:
                                    nc.scalar.dma_start(maskh_t[:], maskh[:])
                                    nc.scalar.dma_start(id_t[:], identd[:])
                                    nc.scalar.dma_start(bob_t[:], bob[:])
                                for sub in range(PAN // P):
                                    ib = ip * (PAN // P) + sub
                                    ps = vpsum.tile([P, FPG], F32, tag="pv")
                                    for kc in range(KC):
                                        nc.tensor.matmul(
                                            ps[:],
                                            xv_t[:, kc, sub * P:(sub + 1) * P],
                                            wv_t[:, kc, :],
                                            start=(kc == 0),
                                            stop=(kc == KC - 1))
                                    nc.vector.tensor_tensor(
                                        vt[:, ib, :], ps[:], bvb_t[:],
                                        op=OP.add)

                        # ---- K/Q projection panel subtasks ----
                        xtiles = {}
                        qpan_tiles = {}

                        def issue_x(kind, p):
                            """Issue the x-panel load for (kind, p) as 4
                            quarter tiles (0.53MB each). Quarter-granular
                            pool rotation keeps the WAR semaphore of each
                            DMA ~2 panels back, so a blocked transfer never
                            head-of-line-blocks the Pool queue for long."""
                            src = xq if kind == "q" else xk
                            qts = []
                            for qc in range(4):
                                ksl = slice(qc * 4, (qc + 1) * 4)
                                qt = xqkp.tile([P, 4, PAN], F32R, tag="xh",
                                               name=f"x{kind}{p}_{qc}")
                                nc.gpsimd.dma_start(qt[:], src[:, p, ksl])
                                qts.append(qt)
                            xtiles[(kind, p)] = qts

                        def emit_panel(kind, p, h):
                            qts = xtiles[(kind, p)]
                            if kind == "q" and p not in qpan_tiles:
                                qpan_tiles[p] = qpanp.tile(
                                    [P, HPG, PAN], F32R, tag="qp",
                                    name=f"qp{p}")
                            w_t = wq_t if kind == "q" else wk_t
                            bias_t = bq_t if kind == "q" else bk_t
                            fsl = slice(h * P, (h + 1) * P)
                            ps = ppsum.tile([P, 512], F32, tag="pp")
                            for kc in range(KC):
                                nc.tensor.matmul(
                                    ps[:, :PAN], w_t[:, kc, fsl],
                                    qts[kc // 4][:, kc % 4, :],
                                    start=(kc == 0), stop=(kc == KC - 1))
                            if kind == "q":
                                dst = qpan_tiles[p][:, h, :]
                            else:
                                dst = kth[:, h, p * PAN:(p + 1) * PAN]
                            nc.scalar.activation(
                                dst, ps[:, :PAN], ACTF.Identity,
                                bias=bias_t[:, h:h + 1], scale=1.0)

                        # prologue: panel 0 of K and Q; x for panels 0-1
                        issue_x("k", 0)
                        issue_x("q", 0)
                        for h in range(HPG):
                            emit_panel("k", 0, h)
                        issue_x("k", 1)
                        for h in range(HPG):
                            emit_panel("q", 0, h)
                        issue_x("q", 1)
                        # panel-2 x rides during group 0 (slots 3 and 7)

                        # ---- phase 2: attention, panel-major ----
                        with tc.tile_pool(name="otpool", bufs=1) as otpool, \
                             tc.tile_pool(name="spsum", bufs=6,
                                          space="PSUM") as spsum, \
                             tc.tile_pool(name="opsum", bufs=1,
                                          space="PSUM") as opsum, \
                             tc.tile_pool(name="ppool", bufs=4) as ppool, \
                             tc.tile_pool(name="ptpool", bufs=4) as ptpool, \
                             tc.tile_pool(name="stats", bufs=8) as stats, \
                             tc.tile_pool(name="dramio", bufs=1,
                                          space="DRAM") as dramio:

                            ots = [otpool.tile([P, L], F16, name=f"ot{hh}")
                                   for hh in range(HPG)]

                            def emit_S(h, ib):
                                nj = (ib + 1) * P
                                widths = _chunk_widths(nj)
                                isl = slice(ib * P, (ib + 1) * P)
                                mpart = stats.tile([P, 4], F32, tag="mp",
                                                   name=f"mp{h}_{ib}")
                                lpart = stats.tile([P, 4], F32, tag="lp",
                                                   name=f"lp{h}_{ib}")
                                qt = qpan_tiles[ib // 2]
                                qsl = qt[:, h,
                                         (ib % 2) * P:(ib % 2 + 1) * P]
                                chunks = []
                                off = 0
                                for jc, w in enumerate(widths):
                                    diag = jc == len(widths) - 1
                                    ps = spsum.tile([P, 512], F32, tag="s",
                                                    name=f"sps{h}_{ib}_{jc}")
                                    nc.tensor.matmul(
                                        ps[:, :w], qsl,
                                        kth[:, h, off:off + w],
                                        start=True, stop=not diag)
                                    if diag:
                                        # causal mask on the diagonal block,
                                        # accumulated on the PE: += I.T @ mask
                                        nc.tensor.matmul(
                                            ps[:, w - P:w], id_t[:],
                                            maskh_t[:],
                                            start=False, stop=True)
                                    chunks.append((ps, w, off, jc))
                                    off += w
                                return mpart, lpart, chunks

                            def emit_S_stats(e):
                                # scores arrive pre-scaled (host folds
                                # sqrt(scale) into Wq/Wk): negated chunk
                                # max IS the exp bias
                                mpart, lpart = e["mpart"], e["lpart"]
                                p_sb = ppool.tile([P, L], F16, tag="p",
                                                  name=f"p{e['h']}_{e['ib']}")
                                e["p_sb"] = p_sb
                                for ps, w, off, jc in e["chunks"]:
                                    nc.vector.reduce_max(
                                        mpart[:, jc:jc + 1], ps[:, :w],
                                        axis=AX.X, negate=True)
                                    nc.scalar.activation(
                                        p_sb[:, off:off + w], ps[:, :w],
                                        ACTF.Exp, bias=mpart[:, jc:jc + 1],
                                        scale=1.0,
                                        accum_out=lpart[:, jc:jc + 1])

                            def emit_scale_tr(e):
                                h, ib, p_sb, mpart, lpart, chunks = (
                                    e["h"], e["ib"], e["p_sb"], e["mpart"],
                                    e["lpart"], e["chunks"])
                                nch = len(chunks)
                                rmin = stats.tile([P, 1], F32, tag="nm",
                                                  name=f"nm{h}_{ib}")
                                nc.vector.tensor_reduce(
                                    rmin[:], mpart[:, :nch], axis=AX.X,
                                    op=OP.min)
                                # per-chunk correction c = exp(m_jc - m)
                                cfac = stats.tile([P, 4], F32, tag="cf",
                                                  name=f"cf{h}_{ib}")
                                nc.scalar.activation(
                                    cfac[:, :nch], mpart[:, :nch],
                                    ACTF.Exp, bias=rmin[:], scale=-1.0)
                                lw = stats.tile([P, 4], F32, tag="lw",
                                                name=f"lw{h}_{ib}")
                                nc.vector.tensor_tensor(
                                    lw[:, :nch], cfac[:, :nch],
                                    lpart[:, :nch], op=OP.mult)
                                lsum = stats.tile([P, 1], F32, tag="ls",
                                                  name=f"ls{h}_{ib}")
                                nc.vector.reduce_sum(lsum[:], lw[:, :nch],
                                                     axis=AX.X)
                                rinv = stats.tile([P, 1], F32, tag="ri",
                                                  name=f"ri{h}_{ib}")
                                nc.vector.reciprocal(rinv[:], lsum[:])
                                # P_jc *= c_jc * rinv, chunks alternating
                                # Pool/DVE so the stage's serial latency
                                # halves (Pool is SBUF-only; DVE runs 4x on
                                # packed fp16)
                                for ps, w, off, jc in chunks:
                                    # late groups: x-load DMAs are done, the
                                    # Pool queue is empty -> all pscales to
                                    # Pool, freeing DVE for the maxes
                                    if ib >= 16:
                                        eng = nc.gpsimd
                                    else:
                                        eng = nc.gpsimd if jc % 2 == 0 \
                                            else nc.vector
                                    eng.tensor_scalar(
                                        p_sb[:, off:off + w],
                                        p_sb[:, off:off + w],
                                        cfac[:, jc:jc + 1], rinv[:],
                                        op0=OP.mult, op1=OP.mult)
                                pt_sb = ptpool.tile([P, IB, P], F16,
                                                    tag="ptsb",
                                                    name=f"ptsb{h}_{ib}")
                                nblk = ib + 1
                                nc.sync.dma_start_transpose(
                                    pt_sb[:, :nblk, :], p_sb[:, :nblk * P])
                                e["pt_sb"] = pt_sb

                            def emit_av(e):
                                h, ib, pt_sb = e["h"], e["ib"], e["pt_sb"]
                                isl = slice(ib * P, (ib + 1) * P)
                                o_ps = opsum.tile([P, P], F32, tag="o",
                                                  name=f"o{h}_{ib}")
                                for jb in range(ib + 1):
                                    nc.tensor.matmul(
                                        o_ps[:],
                                        vt[:, jb, h * P:(h + 1) * P],
                                        pt_sb[:, jb, :],
                                        start=(jb == 0), stop=(jb == ib))
                                nc.vector.tensor_copy(
                                    ots[h][:, isl], o_ps[:])

                            # 4-stage pipeline: S(n) | stats(n-1) |
                            # scale+transpose(n-2) | AV(n-4) — each block's
                            # transpose gets ~2 block-iterations of wall
                            # time before its AV matmuls hit the PE
                            pend = []

                            def push_block(h, ib):
                                # stats/scale of older blocks FIRST: the
                                # spsum/ppool rotations must see those
                                # reader instructions before S(n) reuses
                                # their buffers (pool WAR is emission-order)
                                if len(pend) >= 1:
                                    emit_S_stats(pend[-1])
                                if len(pend) >= 2:
                                    emit_scale_tr(pend[-2])
                                mpart, lpart, chunks = emit_S(h, ib)
                                pend.append(dict(
                                    h=h, ib=ib, mpart=mpart,
                                    lpart=lpart, chunks=chunks))
                                while len(pend) > 5:
                                    emit_av(pend.pop(0))

                            for g in range(NGRP):
                                # late half of panel g's tasks (heads 2-3:
                                # their group-g blocks sit at slots 4-7) +
                                # early half of panel g+1's — spreads proj
                                # work one group later so group 7 isn't
                                # bare of PE filler
                                p = g + 1
                                tasks = []
                                if p < NPAN:
                                    tasks = ([("k", p, hh)
                                              for hh in range(HPG)] +
                                             [("q", p, hh)
                                              for hh in range(HPG)])
                                ti = 0
                                for h in range(HPG):
                                    for sub in range(2):
                                        ib = 2 * g + sub
                                        push_block(h, ib)
                                        if ti < len(tasks):
                                            emit_panel(*tasks[ti])
                                            ti += 1
                                        if h == 1 and sub == 1 \
                                                and g + 2 < NPAN:
                                            issue_x("k", g + 2)
                                        if h == 3 and sub == 1 \
                                                and g + 2 < NPAN:
                                            issue_x("q", g + 2)
                            emit_S_stats(pend[-1])
                            emit_scale_tr(pend[-2])
                            emit_scale_tr(pend[-1])
                            while pend:
                                emit_av(pend.pop(0))

                            # ---- gathers (all heads complete here) ----
                            if variant in ("nocoll", "x2nc"):
                                for h in range(HPG):
                                    ag_out = dramio.tile(
                                        [G, P, L], F16, tag=f"agout{h}",
                                        name=f"agout{h}")
                                    ag_outs.append(ag_out)
                                # seq-quarter-major so the Wo stream's first
                                # slices are ready after 2MB, not 8MB
                                for lq in range(4):
                                    lsl = slice(lq * 512, (lq + 1) * 512)
                                    for h in range(HPG):
                                        for gg in range(G):
                                            nc.sync.dma_start(
                                                ag_outs[h][gg][:, lsl],
                                                ots[h][:, lsl])
                            else:
                                for h in range(HPG):
                                    ag_in = dramio.tile(
                                        [P, L], F16, tag=f"agin{h}",
                                        name=f"agin{h}")
                                    nc.scalar.dma_start(ag_in[:], ots[h][:])
                                    ag_out = dramio.tile(
                                        [G, P, L], F16, tag=f"agout{h}",
                                        name=f"agout{h}")
                                    nc.gpsimd.collective_compute(
                                        "AllGather", OP.bypass,
                                        replica_groups=[[0, 1, 2, 3],
                                                        [4, 5, 6, 7]],
                                        ins=[ag_in[:].opt()],
                                        outs=[ag_out[:].opt()])
                                    ag_outs.append(ag_out)

                    # ---- phase 3: output projection, streamed in 256-seq
                    # slices (512B descriptors -> full DMA rate) ----
                    with tc.tile_pool(name="ph3", bufs=1) as ph3, \
                         tc.tile_pool(name="fapool", bufs=4) as fapool, \
                         tc.tile_pool(name="fopool", bufs=4) as fopool, \
                         tc.tile_pool(name="fpsum", bufs=8,
                                      space="PSUM") as fpsum:
                        wo_t = ph3.tile([P, KC, FPG], F16, name=f"wo{_rep}")
                        nc.scalar.dma_start(wo_t[:, :KC // 2],
                                            woT[:, :KC // 2])
                        nc.scalar.dma_start(wo_t[:, KC // 2:],
                                            woT[:, KC // 2:])
                        NQ = 8
                        QW = L // NQ   # 256 seq cols per slice
                        for q in range(NQ):
                            qsl = slice(q * QW, (q + 1) * QW)
                            atq = fapool.tile([P, HPG, G, QW], F16,
                                              tag="atq", name=f"atq{q}")
                            for hc in range(HPG):
                                nc.scalar.dma_start(
                                    atq[:, hc],
                                    ag_outs[hc].rearrange(
                                        "g p l -> p g l")[:, :, qsl])
                            pss = [fpsum.tile([P, FPG], F32, tag="f",
                                              name=f"fps{q}_{i}")
                                   for i in range(2)]
                            for hc in range(HPG):
                                for g_idx in range(G):
                                    for i in range(2):
                                        nc.tensor.matmul(
                                            pss[i][:],
                                            atq[:, hc, g_idx,
                                                i * P:(i + 1) * P],
                                            wo_t[:, g_idx * HPG + hc, :],
                                            start=(hc == 0 and g_idx == 0),
                                            stop=(hc == HPG - 1
                                                  and g_idx == G - 1))
                            for i in range(2):
                                ib = 2 * q + i
                                o_sb = fopool.tile([P, FPG], F32, tag="fo")
                                nc.vector.tensor_tensor(
                                    o_sb[:], pss[i][:], bob_t[:], op=OP.add)
                                nc.gpsimd.dma_start(
                                    out[ib * P:(ib + 1) * P, :], o_sb[:])

    nc.compile()
    return nc


def _tilex(x, dtype):
    # [D, L] -> [P, NPAN, KC, PAN]: (kc*128+p, pl*PAN+c) -> [p, pl, kc, c]
    return np.ascontiguousarray(
        x.reshape(KC, P, NPAN, PAN).transpose(1, 2, 0, 3).astype(dtype))


def _tilew(w, dtype):
    # [D, FPG] -> [P, KC, FPG]
    return np.ascontiguousarray(
        w.reshape(KC, P, FPG).transpose(1, 0, 2).astype(dtype))


def _prepare_in_maps(q, k, v, Wq, bq, Wk, bk, Wv, bv, Wo, bo):
    mask16 = np.where(
        np.arange(P)[None, :] > np.arange(P)[:, None],
        np.float16(-30000.0), np.float16(0.0)).astype(np.float16)
    ident = np.eye(P, dtype=np.float16)

    xs = {}
    for b in range(B):
        for nm, arr in (("q", q), ("k", k)):
            xs[(nm, b)] = _tilex(
                np.ascontiguousarray(arr[b].T, dtype=np.float32), np.float32)
        xs[("v", b)] = _tilex(
            np.ascontiguousarray(v[b].T, dtype=np.float32), np.float16)

    rs = np.float32(SCALE ** 0.5)
    in_maps = []
    for c in range(8):
        b, g = divmod(c, G)
        F = slice(g * FPG, (g + 1) * FPG)
        in_maps.append({
            "xq": xs[("q", b)],
            "xk": xs[("k", b)],
            "xv": xs[("v", b)],
            "wq": _tilew(
                np.ascontiguousarray(Wq[F, :].T, dtype=np.float32) * rs,
                np.float32),
            "wk": _tilew(
                np.ascontiguousarray(Wk[F, :].T, dtype=np.float32) * rs,
                np.float32),
            "wv": _tilew(
                np.ascontiguousarray(Wv[F, :].T, dtype=np.float32),
                np.float16),
            "woT": _tilew(
                np.ascontiguousarray(Wo[F, :].T, dtype=np.float32),
                np.float16),
            "bq": np.ascontiguousarray(
                (bq[F] * rs).astype(np.float32).reshape(HPG, P).T),
            "bk": np.ascontiguousarray(
                (bk[F] * rs).astype(np.float32).reshape(HPG, P).T),
            "bvb": np.broadcast_to(bv[F][None, :], (P, FPG)).astype(
                np.float32),
            "bob": np.broadcast_to(bo[F][None, :], (P, FPG)).astype(
                np.float32),
            "maskh": mask16,
            "identd": ident,
        })
    return in_maps


def kernel(**inputs) -> np.ndarray:
    global _COMPILED
    from concourse.bass_utils import run_bass_kernel_spmd

    if _COMPILED is None:
        _COMPILED = _build()
    nc = _COMPILED

    in_maps = _prepare_in_maps(**inputs)
    res = run_bass_kernel_spmd(nc, in_maps, list(range(8)))

    outp = np.empty((B, L, D), dtype=np.float32)
    for c in range(8):
        b, g = divmod(c, G)
        outp[b, :, g * FPG:(g + 1) * FPG] = res.results[c]["out"]
    return outp


if __name__ == "__main__":
    rng = np.random.default_rng(1)
    ins = {
        "q": rng.standard_normal((B, L, D), dtype=np.float32),
        "k": rng.standard_normal((B, L, D), dtype=np.float32),
        "v": rng.standard_normal((B, L, D), dtype=np.float32),
        "Wq": rng.standard_normal((D, D), dtype=np.float32) * 0.02,
        "bq": rng.standard_normal(D).astype(np.float32) * 0.02,
        "Wk": rng.standard_normal((D, D), dtype=np.float32) * 0.02,
        "bk": rng.standard_normal(D).astype(np.float32) * 0.02,
        "Wv": rng.standard_normal((D, D), dtype=np.float32) * 0.02,
        "bv": rng.standard_normal(D).astype(np.float32) * 0.02,
        "Wo": rng.standard_normal((D, D), dtype=np.float32) * 0.02,
        "bo": rng.standard_normal(D).astype(np.float32) * 0.02,
    }
    o = kernel(**ins)
    print("kernel ran, out shape", o.shape)


# revision 57
# speedup vs baseline: 1.1089x; 1.0265x over previous
"""Trainium2 Bass kernel for causal multi-head attention (B=2, L=2048, D=2048,
H=16 heads, DH=128), sharded over 8 NeuronCores.

Sharding: core c handles batch b=c//4 and head-group g=c%4 (4 heads = 512
features). The only cross-core communication is a per-head fp16 AllGather of
attention outputs within each 4-core batch group.

v3 design (fp32r + panel-major interleave):
- Q/K projections and the score matmuls run in float32r: the PE processes
  f32r at 1 col/cycle when the moving dim is >= 256 (same as fp16), while
  storage rounds to ~14 mantissa bits — accurate enough to drop the whole
  compensated fp16+fp8 DoubleRow correction scheme of v2 (S abs err ~0.02 vs
  the 0.109 budget).
- Attention runs PANEL-MAJOR: groups g=0..7 process seq blocks {2g, 2g+1} of
  all 4 heads. K/Q projection panel subtasks (one head x one 256-seq panel,
  16 matmuls) interleave one-per-block into group g for panel g+1, so the PE
  always has dense matmul work while the softmax chain (DVE max, ACT exp,
  Pool pscale, DMA transpose) drains. qth lives as 2 rotating 256-seq panel
  tiles; kth accumulates in a full 4MB f32r tile.
- Engine split: DVE = chunk maxes + stats; ACT = exp (accum_out row sums) +
  proj/o evacuations; Pool = pscales (Pool cannot touch PSUM); PE adds the
  causal mask via the id^T @ mask accumulation trick.
- DMA queues: sync = V-phase loads + P^T transposes + nocoll gather copies;
  scalar = consts + wq/wk prefetch + ag bounce + wo + atq; vector = xq/xk
  half-tile streams + out writes. Keeps the latency-critical transposes from
  queuing behind bulk loads.
- Wo phase reads gathered activations in 256-seq slices so every descriptor
  is 512B (full DMA rate; 128-seq slices pay the sub-512B 2x penalty).
- PSUM: 2 proj banks + 4 score banks + 2 O^T banks (each O^T accumulator
  must own a bank: concurrent groups sharing a bank corrupt via bank-granular
  has_written clears). Score chunks use widths >=256 (e.g. 384+256 instead
  of 512+128) to stay at the f32r fast rate.
"""
import sys

sys.path.insert(0, "/opt/trn_rl_repo")

import numpy as np

B, L, D, H = 2, 2048, 2048, 16
DH = D // H          # 128
G = 4                # head-groups (tensor-parallel degree per batch)
HPG = H // G         # heads per group = 4
FPG = HPG * DH       # features per group = 512
P = 128
SCALE = float(DH) ** 0.5
KC = D // P          # 16 contraction chunks
IB = L // P          # 16 seq blocks of 128
PAN = 256            # K/Q/V projection seq panel
NPAN = L // PAN      # 8
NGRP = IB // 2       # 8 block groups (2 blocks x 4 heads each)

_COMPILED = None


def _chunk_widths(nj):
    """Split nj into chunks of <=512 with every chunk >=256 when possible
    (f32r matmuls run 4x slower below a 256-wide moving dim)."""
    if nj <= 512:
        return [nj]
    nch = (nj + 511) // 512
    rem = nj - 512 * (nch - 1)
    w = [512] * (nch - 1) + [rem]
    if rem < 256:
        w[-2:] = [512 + rem - 256, 256]
    return w


def _build(variant="main"):
    import concourse.bacc as bacc
    import concourse.tile as tile
    from concourse import mybir
    from contextlib import ExitStack

    F32 = mybir.dt.float32
    F32R = mybir.dt.float32r
    F16 = mybir.dt.float16
    AX = mybir.AxisListType
    OP = mybir.AluOpType
    ACTF = mybir.ActivationFunctionType

    nc = bacc.Bacc("TRN2", target_bir_lowering=False, debug=False, num_devices=8)

    # ---- DRAM I/O (x/w pre-arranged host-side in tile order) ----
    xq = nc.dram_tensor("xq", [P, NPAN, KC, PAN], F32R, kind="ExternalInput")
    xk = nc.dram_tensor("xk", [P, NPAN, KC, PAN], F32R, kind="ExternalInput")
    xv = nc.dram_tensor("xv", [P, NPAN, KC, PAN], F16, kind="ExternalInput")
    wq = nc.dram_tensor("wq", [P, KC, FPG], F32R, kind="ExternalInput")
    wk = nc.dram_tensor("wk", [P, KC, FPG], F32R, kind="ExternalInput")
    wv = nc.dram_tensor("wv", [P, KC, FPG], F16, kind="ExternalInput")
    woT = nc.dram_tensor("woT", [P, KC, FPG], F16, kind="ExternalInput")
    bq = nc.dram_tensor("bq", [P, HPG], F32, kind="ExternalInput")
    bk = nc.dram_tensor("bk", [P, HPG], F32, kind="ExternalInput")
    bvb = nc.dram_tensor("bvb", [P, FPG], F32, kind="ExternalInput")
    bob = nc.dram_tensor("bob", [P, FPG], F32, kind="ExternalInput")
    maskh = nc.dram_tensor("maskh", [P, P], F16, kind="ExternalInput")
    identd = nc.dram_tensor("identd", [P, P], F16, kind="ExternalInput")
    out = nc.dram_tensor("out", [L, FPG], F32, kind="ExternalOutput")
    if variant == "timing":
        chain = nc.dram_tensor("chain", [1, 8], F32, kind="ExternalInput")
        dummy = nc.dram_tensor("chaino", [1, 8], F32, kind="ExternalOutput")

    with tile.TileContext(nc) as tc:
        with ExitStack() as ctx:
            consts = ctx.enter_context(tc.tile_pool(name="consts", bufs=1))

            maskh_t = consts.tile([P, P], F16)
            id_t = consts.tile([P, P], F16)
            bq_t = consts.tile([P, HPG], F32)
            bk_t = consts.tile([P, HPG], F32)
            bvb_t = consts.tile([P, FPG], F32)
            nc.scalar.dma_start(bvb_t[:], bvb[:])
            nc.scalar.dma_start(bq_t[:], bq[:])
            nc.scalar.dma_start(bk_t[:], bk[:])
            bob_t = consts.tile([P, FPG], F32)
            if variant == "timing":
                ch_t = consts.tile([1, 8], F32)
                nc.sync.dma_start(ch_t[:], chain[:])
                nc.sync.dma_start(dummy[:], ch_t[:])

            NREP = {"x4": 4, "x2": 2, "x2nc": 2}.get(variant, 1)
            for _rep in range(NREP):
                ag_outs = []
                with tc.tile_pool(name="qkv", bufs=1) as qkv:
                    kth = qkv.tile([P, HPG, L], F32R)   # (dh, head, j)
                    vt = qkv.tile([P, IB, FPG], F16)    # (j%128, blk, feat)

                    with tc.tile_pool(name="wqk", bufs=1) as wqk, \
                         tc.tile_pool(name="xqk", bufs=8) as xqkp, \
                         tc.tile_pool(name="qpan", bufs=2) as qpanp, \
                         tc.tile_pool(name="ppsum", bufs=1,
                                      space="PSUM") as ppsum:
                        wq_t = wqk.tile([P, KC, FPG], F32R)
                        wk_t = wqk.tile([P, KC, FPG], F32R)

                        # ---- phase 1: V projection (fp16) ----
                        with tc.tile_pool(name="wvp", bufs=1) as wvp, \
                             tc.tile_pool(name="xvp", bufs=2) as xvp, \
                             tc.tile_pool(name="vpsum", bufs=2,
                                          space="PSUM") as vpsum:
                            wv_t = wvp.tile([P, KC, FPG], F16)
                            for ip in range(NPAN):
                                xv_t = xvp.tile([P, KC, PAN], F16, tag="xv")
                                if ip == 0:
                                    # interleave w/x quarters so the first
                                    # matmul's operands land first
                                    for xc in range(4):
                                        ksl = slice(xc * 4, xc * 4 + 4)
                                        nc.sync.dma_start(xv_t[:, ksl],
                                                          xv[:, ip, ksl])
                                        nc.sync.dma_start(
                                            wv_t[:, ksl], wv[:, ksl])
                                else:
                                    nc.sync.dma_start(xv_t[:], xv[:, ip])
                                if 1 <= ip <= 4:
                                    # prefetch K/Q weights on the scalar
                                    # queue, quartered so xv panels interleave
                                    # at the (serial) DMA transfer resource
                                    ksl = slice((ip - 1) * 4, ip * 4)
                                    nc.scalar.dma_start(wk_t[:, ksl],
                                                        wk[:, ksl])
                                if 3 <= ip <= 6:
                                    ksl = slice((ip - 3) * 4, (ip - 2) * 4)
                                    nc.scalar.dma_start(wq_t[:, ksl],
                                                        wq[:, ksl])
                                if ip == 5 and _rep == 0:
                                    nc.scalar.dma_start(maskh_t[:], maskh[:])
                                    nc.scalar.dma_start(id_t[:], identd[:])
                                    nc.scalar.dma_start(bob_t[:], bob[:])
                                for sub in range(PAN // P):
                                    ib = ip * (PAN // P) + sub
                                    ps = vpsum.tile([P, FPG], F32, tag="pv")
                                    for kc in range(KC):
                                        nc.tensor.matmul(
                                            ps[:],
                                            xv_t[:, kc, sub * P:(sub + 1) * P],
                                            wv_t[:, kc, :],
                                            start=(kc == 0),
                                            stop=(kc == KC - 1))
                                    nc.vector.tensor_tensor(
                                        vt[:, ib, :], ps[:], bvb_t[:],
                                        op=OP.add)

                        # ---- K/Q projection panel subtasks ----
                        xtiles = {}
                        qpan_tiles = {}

                        def issue_x(kind, p):
                            """Issue the x-panel load for (kind, p) as 4
                            quarter tiles (0.53MB each). Quarter-granular
                            pool rotation keeps the WAR semaphore of each
                            DMA ~2 panels back, so a blocked transfer never
                            head-of-line-blocks the Pool queue for long."""
                            src = xq if kind == "q" else xk
                            qts = []
                            for qc in range(4):
                                ksl = slice(qc * 4, (qc + 1) * 4)
                                qt = xqkp.tile([P, 4, PAN], F32R, tag="xh",
                                               name=f"x{kind}{p}_{qc}")
                                nc.gpsimd.dma_start(qt[:], src[:, p, ksl])
                                qts.append(qt)
                            xtiles[(kind, p)] = qts

                        def emit_panel(kind, p, h):
                            qts = xtiles[(kind, p)]
                            if kind == "q" and p not in qpan_tiles:
                                qpan_tiles[p] = qpanp.tile(
                                    [P, HPG, PAN], F32R, tag="qp",
                                    name=f"qp{p}")
                            w_t = wq_t if kind == "q" else wk_t
                            bias_t = bq_t if kind == "q" else bk_t
                            fsl = slice(h * P, (h + 1) * P)
                            ps = ppsum.tile([P, 512], F32, tag="pp")
                            for kc in range(KC):
                                nc.tensor.matmul(
                                    ps[:, :PAN], w_t[:, kc, fsl],
                                    qts[kc // 4][:, kc % 4, :],
                                    start=(kc == 0), stop=(kc == KC - 1))
                            if kind == "q":
                                nc.scalar.activation(
                                    qpan_tiles[p][:, h, :], ps[:, :PAN],
                                    ACTF.Identity,
                                    bias=bias_t[:, h:h + 1], scale=1.0)
                            else:
                                nc.vector.tensor_scalar(
                                    kth[:, h, p * PAN:(p + 1) * PAN],
                                    ps[:, :PAN], bias_t[:, h:h + 1], None,
                                    op0=OP.add)

                        # prologue: panel 0 of K and Q; x for panels 0-1
                        issue_x("k", 0)
                        issue_x("q", 0)
                        for h in range(HPG):
                            emit_panel("k", 0, h)
                        issue_x("k", 1)
                        for h in range(HPG):
                            emit_panel("q", 0, h)
                        issue_x("q", 1)
                        # panel-2 x rides during group 0 (slots 3 and 7)

                        # ---- phase 2: attention, panel-major ----
                        with tc.tile_pool(name="otpool", bufs=1) as otpool, \
                             tc.tile_pool(name="spsum", bufs=6,
                                          space="PSUM") as spsum, \
                             tc.tile_pool(name="opsum", bufs=1,
                                          space="PSUM") as opsum, \
                             tc.tile_pool(name="ppool", bufs=4) as ppool, \
                             tc.tile_pool(name="ptpool", bufs=4) as ptpool, \
                             tc.tile_pool(name="stats", bufs=8) as stats, \
                             tc.tile_pool(name="dramio", bufs=1,
                                          space="DRAM") as dramio:

                            ots = [otpool.tile([P, L], F16, name=f"ot{hh}")
                                   for hh in range(HPG)]

                            def emit_S(h, ib):
                                nj = (ib + 1) * P
                                widths = _chunk_widths(nj)
                                isl = slice(ib * P, (ib + 1) * P)
                                mpart = stats.tile([P, 4], F32, tag="mp",
                                                   name=f"mp{h}_{ib}")
                                lpart = stats.tile([P, 4], F32, tag="lp",
                                                   name=f"lp{h}_{ib}")
                                qt = qpan_tiles[ib // 2]
                                qsl = qt[:, h,
                                         (ib % 2) * P:(ib % 2 + 1) * P]
                                chunks = []
                                off = 0
                                for jc, w in enumerate(widths):
                                    diag = jc == len(widths) - 1
                                    ps = spsum.tile([P, 512], F32, tag="s",
                                                    name=f"sps{h}_{ib}_{jc}")
                                    nc.tensor.matmul(
                                        ps[:, :w], qsl,
                                        kth[:, h, off:off + w],
                                        start=True, stop=not diag)
                                    if diag:
                                        # causal mask on the diagonal block,
                                        # accumulated on the PE: += I.T @ mask
                                        nc.tensor.matmul(
                                            ps[:, w - P:w], id_t[:],
                                            maskh_t[:],
                                            start=False, stop=True)
                                    chunks.append((ps, w, off, jc))
                                    off += w
                                return mpart, lpart, chunks

                            def emit_S_stats(e):
                                # scores arrive pre-scaled (host folds
                                # sqrt(scale) into Wq/Wk): negated chunk
                                # max IS the exp bias
                                mpart, lpart = e["mpart"], e["lpart"]
                                p_sb = ppool.tile([P, L], F16, tag="p",
                                                  name=f"p{e['h']}_{e['ib']}")
                                e["p_sb"] = p_sb
                                for ps, w, off, jc in e["chunks"]:
                                    nc.vector.reduce_max(
                                        mpart[:, jc:jc + 1], ps[:, :w],
                                        axis=AX.X, negate=True)
                                    nc.scalar.activation(
                                        p_sb[:, off:off + w], ps[:, :w],
                                        ACTF.Exp, bias=mpart[:, jc:jc + 1],
                                        scale=1.0,
                                        accum_out=lpart[:, jc:jc + 1])

                            def emit_scale_tr(e):
                                h, ib, p_sb, mpart, lpart, chunks = (
                                    e["h"], e["ib"], e["p_sb"], e["mpart"],
                                    e["lpart"], e["chunks"])
                                nch = len(chunks)
                                rmin = stats.tile([P, 1], F32, tag="nm",
                                                  name=f"nm{h}_{ib}")
                                nc.vector.tensor_reduce(
                                    rmin[:], mpart[:, :nch], axis=AX.X,
                                    op=OP.min)
                                # per-chunk correction c = exp(m_jc - m)
                                cfac = stats.tile([P, 4], F32, tag="cf",
                                                  name=f"cf{h}_{ib}")
                                nc.scalar.activation(
                                    cfac[:, :nch], mpart[:, :nch],
                                    ACTF.Exp, bias=rmin[:], scale=-1.0)
                                lw = stats.tile([P, 4], F32, tag="lw",
                                                name=f"lw{h}_{ib}")
                                nc.vector.tensor_tensor(
                                    lw[:, :nch], cfac[:, :nch],
                                    lpart[:, :nch], op=OP.mult)
                                lsum = stats.tile([P, 1], F32, tag="ls",
                                                  name=f"ls{h}_{ib}")
                                nc.vector.reduce_sum(lsum[:], lw[:, :nch],
                                                     axis=AX.X)
                                rinv = stats.tile([P, 1], F32, tag="ri",
                                                  name=f"ri{h}_{ib}")
                                nc.vector.reciprocal(rinv[:], lsum[:])
                                # P_jc *= c_jc * rinv, chunks alternating
                                # Pool/DVE so the stage's serial latency
                                # halves (Pool is SBUF-only; DVE runs 4x on
                                # packed fp16)
                                for ps, w, off, jc in chunks:
                                    # late groups: x-load DMAs are done, the
                                    # Pool queue is empty -> all pscales to
                                    # Pool, freeing DVE for the maxes
                                    if ib >= 16:
                                        eng = nc.gpsimd
                                    else:
                                        eng = nc.gpsimd if jc % 2 == 0 \
                                            else nc.vector
                                    eng.tensor_scalar(
                                        p_sb[:, off:off + w],
                                        p_sb[:, off:off + w],
                                        cfac[:, jc:jc + 1], rinv[:],
                                        op0=OP.mult, op1=OP.mult)
                                pt_sb = ptpool.tile([P, IB, P], F16,
                                                    tag="ptsb",
                                                    name=f"ptsb{h}_{ib}")
                                nblk = ib + 1
                                nc.sync.dma_start_transpose(
                                    pt_sb[:, :nblk, :], p_sb[:, :nblk * P])
                                e["pt_sb"] = pt_sb

                            def emit_av(e):
                                h, ib, pt_sb = e["h"], e["ib"], e["pt_sb"]
                                isl = slice(ib * P, (ib + 1) * P)
                                o_ps = opsum.tile([P, P], F32, tag="o",
                                                  name=f"o{h}_{ib}")
                                for jb in range(ib + 1):
                                    nc.tensor.matmul(
                                        o_ps[:],
                                        vt[:, jb, h * P:(h + 1) * P],
                                        pt_sb[:, jb, :],
                                        start=(jb == 0), stop=(jb == ib))
                                nc.vector.tensor_copy(
                                    ots[h][:, isl], o_ps[:])

                            # 4-stage pipeline: S(n) | stats(n-1) |
                            # scale+transpose(n-2) | AV(n-4) — each block's
                            # transpose gets ~2 block-iterations of wall
                            # time before its AV matmuls hit the PE
                            pend = []

                            def push_block(h, ib):
                                # stats/scale of older blocks FIRST: the
                                # spsum/ppool rotations must see those
                                # reader instructions before S(n) reuses
                                # their buffers (pool WAR is emission-order)
                                if len(pend) >= 1:
                                    emit_S_stats(pend[-1])
                                if len(pend) >= 2:
                                    emit_scale_tr(pend[-2])
                                mpart, lpart, chunks = emit_S(h, ib)
                                pend.append(dict(
                                    h=h, ib=ib, mpart=mpart,
                                    lpart=lpart, chunks=chunks))
                                while len(pend) > 5:
                                    emit_av(pend.pop(0))

                            for g in range(NGRP):
                                # late half of panel g's tasks (heads 2-3:
                                # their group-g blocks sit at slots 4-7) +
                                # early half of panel g+1's — spreads proj
                                # work one group later so group 7 isn't
                                # bare of PE filler
                                p = g + 1
                                tasks = []
                                if p < NPAN:
                                    tasks = ([("k", p, hh)
                                              for hh in range(HPG)] +
                                             [("q", p, hh)
                                              for hh in range(HPG)])
                                ti = 0
                                for h in range(HPG):
                                    for sub in range(2):
                                        ib = 2 * g + sub
                                        push_block(h, ib)
                                        if ti < len(tasks):
                                            emit_panel(*tasks[ti])
                                            ti += 1
                                        if h == 1 and sub == 1 \
                                                and g + 2 < NPAN:
                                            issue_x("k", g + 2)
                                        if h == 3 and sub == 1 \
                                                and g + 2 < NPAN:
                                            issue_x("q", g + 2)
                            emit_S_stats(pend[-1])
                            emit_scale_tr(pend[-2])
                            emit_scale_tr(pend[-1])
                            while pend:
                                emit_av(pend.pop(0))

                            # ---- gathers (all heads complete here) ----
                            if variant in ("nocoll", "x2nc"):
                                for h in range(HPG):
                                    ag_out = dramio.tile(
                                        [G, P, L], F16, tag=f"agout{h}",
                                        name=f"agout{h}")
                                    ag_outs.append(ag_out)
                                # seq-quarter-major so the Wo stream's first
                                # slices are ready after 2MB, not 8MB
                                for lq in range(4):
                                    lsl = slice(lq * 512, (lq + 1) * 512)
                                    for h in range(HPG):
                                        for gg in range(G):
                                            nc.sync.dma_start(
                                                ag_outs[h][gg][:, lsl],
                                                ots[h][:, lsl])
                            else:
                                for h in range(HPG):
                                    ag_in = dramio.tile(
                                        [P, L], F16, tag=f"agin{h}",
                                        name=f"agin{h}")
                                    nc.scalar.dma_start(ag_in[:], ots[h][:])
                                    ag_out = dramio.tile(
                                        [G, P, L], F16, tag=f"agout{h}",
                                        name=f"agout{h}")
                                    nc.gpsimd.collective_compute(
                                        "AllGather", OP.bypass,
                                        replica_groups=[[0, 1, 2, 3],
                                                        [4, 5, 6, 7]],
                                        ins=[ag_in[:].opt()],
                                        outs=[ag_out[:].opt()])
                                    ag_outs.append(ag_out)

                    # ---- phase 3: output projection, streamed in 256-seq
                    # slices (512B descriptors -> full DMA rate) ----
                    with tc.tile_pool(name="ph3", bufs=1) as ph3, \
                         tc.tile_pool(name="fapool", bufs=4) as fapool, \
                         tc.tile_pool(name="fopool", bufs=4) as fopool, \
                         tc.tile_pool(name="fpsum", bufs=8,
                                      space="PSUM") as fpsum:
                        wo_t = ph3.tile([P, KC, FPG], F16, name=f"wo{_rep}")
                        nc.scalar.dma_start(wo_t[:, :KC // 2],
                                            woT[:, :KC // 2])
                        nc.scalar.dma_start(wo_t[:, KC // 2:],
                                            woT[:, KC // 2:])
                        NQ = 8
                        QW = L // NQ   # 256 seq cols per slice
                        for q in range(NQ):
                            qsl = slice(q * QW, (q + 1) * QW)
                            atq = fapool.tile([P, HPG, G, QW], F16,
                                              tag="atq", name=f"atq{q}")
                            for hc in range(HPG):
                                nc.scalar.dma_start(
                                    atq[:, hc],
                                    ag_outs[hc].rearrange(
                                        "g p l -> p g l")[:, :, qsl])
                            pss = [fpsum.tile([P, FPG], F32, tag="f",
                                              name=f"fps{q}_{i}")
                                   for i in range(2)]
                            for hc in range(HPG):
                                for g_idx in range(G):
                                    for i in range(2):
                                        nc.tensor.matmul(
                                            pss[i][:],
                                            atq[:, hc, g_idx,
                                                i * P:(i + 1) * P],
                                            wo_t[:, g_idx * HPG + hc, :],
                                            start=(hc == 0 and g_idx == 0),
                                            stop=(hc == HPG - 1
                                                  and g_idx == G - 1))
                            for i in range(2):
                                ib = 2 * q + i
                                o_sb = fopool.tile([P, FPG], F32, tag="fo")
                                nc.vector.tensor_tensor(
                                    o_sb[:], pss[i][:], bob_t[:], op=OP.add)
                                nc.gpsimd.dma_start(
                                    out[ib * P:(ib + 1) * P, :], o_sb[:])

    nc.compile()
    return nc


def _tilex(x, dtype):
    # [D, L] -> [P, NPAN, KC, PAN]: (kc*128+p, pl*PAN+c) -> [p, pl, kc, c]
    return np.ascontiguousarray(
        x.reshape(KC, P, NPAN, PAN).transpose(1, 2, 0, 3).astype(dtype))


def _tilew(w, dtype):
    # [D, FPG] -> [P, KC, FPG]
    return np.ascontiguousarray(
        w.reshape(KC, P, FPG).transpose(1, 0, 2).astype(dtype))


def _prepare_in_maps(q, k, v, Wq, bq, Wk, bk, Wv, bv, Wo, bo):
    mask16 = np.where(
        np.arange(P)[None, :] > np.arange(P)[:, None],
        np.float16(-30000.0), np.float16(0.0)).astype(np.float16)
    ident = np.eye(P, dtype=np.float16)

    xs = {}
    for b in range(B):
        for nm, arr in (("q", q), ("k", k)):
            xs[(nm, b)] = _tilex(
                np.ascontiguousarray(arr[b].T, dtype=np.float32), np.float32)
        xs[("v", b)] = _tilex(
            np.ascontiguousarray(v[b].T, dtype=np.float32), np.float16)

    rs = np.float32(SCALE ** 0.5)
    in_maps = []
    for c in range(8):
        b, g = divmod(c, G)
        F = slice(g * FPG, (g + 1) * FPG)
        in_maps.append({
            "xq": xs[("q", b)],
            "xk": xs[("k", b)],
            "xv": xs[("v", b)],
            "wq": _tilew(
                np.ascontiguousarray(Wq[F, :].T, dtype=np.float32) * rs,
                np.float32),
            "wk": _tilew(
                np.ascontiguousarray(Wk[F, :].T, dtype=np.float32) * rs,
                np.float32),
            "wv": _tilew(
                np.ascontiguousarray(Wv[F, :].T, dtype=np.float32),
                np.float16),
            "woT": _tilew(
                np.ascontiguousarray(Wo[F, :].T, dtype=np.float32),
                np.float16),
            "bq": np.ascontiguousarray(
                (bq[F] * rs).astype(np.float32).reshape(HPG, P).T),
            "bk": np.ascontiguousarray(
                (bk[F] * rs).astype(np.float32).reshape(HPG, P).T),
            "bvb": np.broadcast_to(bv[F][None, :], (P, FPG)).astype(
                np.float32),
            "bob": np.broadcast_to(bo[F][None, :], (P, FPG)).astype(
                np.float32),
            "maskh": mask16,
            "identd": ident,
        })
    return in_maps


def kernel(**inputs) -> np.ndarray:
    global _COMPILED
    from concourse.bass_utils import run_bass_kernel_spmd

    if _COMPILED is None:
        _COMPILED = _build()
    nc = _COMPILED

    in_maps = _prepare_in_maps(**inputs)
    res = run_bass_kernel_spmd(nc, in_maps, list(range(8)))

    outp = np.empty((B, L, D), dtype=np.float32)
    for c in range(8):
        b, g = divmod(c, G)
        outp[b, :, g * FPG:(g + 1) * FPG] = res.results[c]["out"]
    return outp


if __name__ == "__main__":
    rng = np.random.default_rng(1)
    ins = {
        "q": rng.standard_normal((B, L, D), dtype=np.float32),
        "k": rng.standard_normal((B, L, D), dtype=np.float32),
        "v": rng.standard_normal((B, L, D), dtype=np.float32),
        "Wq": rng.standard_normal((D, D), dtype=np.float32) * 0.02,
        "bq": rng.standard_normal(D).astype(np.float32) * 0.02,
        "Wk": rng.standard_normal((D, D), dtype=np.float32) * 0.02,
        "bk": rng.standard_normal(D).astype(np.float32) * 0.02,
        "Wv": rng.standard_normal((D, D), dtype=np.float32) * 0.02,
        "bv": rng.standard_normal(D).astype(np.float32) * 0.02,
        "Wo": rng.standard_normal((D, D), dtype=np.float32) * 0.02,
        "bo": rng.standard_normal(D).astype(np.float32) * 0.02,
    }
    o = kernel(**ins)
    print("kernel ran, out shape", o.shape)
